# revision 42
# baseline (speedup 1.0000x reference)
"""Trainium2 Bass kernel: 8-layer ternary (BitNet-1.58) dense transformer.

Model (per reference):
    h = embed[input_ids]                                  # (B=2, S=1024, H=2048)
    8x: y = h @ ternary(W_l)^T + b_l ; h = LN(y + h)*g+b  # H=2048
    h = LN(h)*final_g + final_b
    logits = h @ ternary(head_W)^T                        # (B, S, V=32000)

Sharding over 8 NeuronCores (fully local, no collectives):
  - Layers: data-parallel over the 2048 tokens (256 tokens/core). Each core
    streams the full ternary layer weights as exact {-1,0,+1} fp8(e4m3).
  - Head: ALSO data-parallel over tokens: each core computes its own 256
    tokens x the full 32000-entry vocab, streaming fp8 head weights
    chunk-by-chunk, overlapped with compute. No collectives at all.

Head matmul runs mixed precision: k-tiles 0..DRKT-1 via fp8 DoubleRow (2
k-tiles per instruction, activations rounded to e4m3), the rest as bf16
activations x fp8 weights at full precision. DRKT=8 costs ~1.91e-2 relative
error on the logits (vs the 2e-2 budget; host-simulated AND confirmed on HW)
and saves 2 of 13 matmul slots per vocab chunk vs DRKT=6.

The vocab is NOT padded on the compute side: 62 chunks of 512 plus one tail
chunk of 256 (32000 = 62*512 + 256), saving the 256 dead columns.

Schedule notes (from perfetto/NTFF analysis of the 640us baseline):
  - The PE stream is otherwise dense; the bubbles were (a) per-layer-tile
    ~0.8us waits of the activation transposes on the LayerNorm vector chain
    and (b) a 6.2us layers->head transition stall. Both are fixed by
    software-pipelining the transposes one stage deeper: the transpose
    feeding PE block N+1 is emitted right after block N's matmuls (and
    before the current LN's ScalarE squares, so its drain copies aren't
    queued behind them); ~14us of matmuls cover the ~6us LN chain latency.
    The first drain copy is kt0-only so the next block's first LDWEIGHTS
    unblocks ~250ns earlier.
  - Head transition: T_head(t0) is emitted inside the last layer;
    then q=0..NWARM-1 groups for t0 (covering t1's final LN chain),
    T_head(t1), its warm groups, then the steady loop. Head weight DMA for
    the warm chunks is issued before the last layer's matmuls.
  - The LN row-sum runs per 512-chunk right behind each z = psum + resid
    chunk (replacing one 2.3us full-row reduce). NB: DVE
    InstTensorTensorReduce faults on HW via this toolchain (both with PSUM
    and SBUF inputs) despite passing CoreSim -- do not use it.
  - bf16 logits + bf16 h0 keep the head phase under the per-core DMA
    ceiling (fp8 weight stream at ~200 GB/s/core + output writes); with
    fp32 logits the weight stream starves and the PE idles ~1.5us every
    few vocab chunks.
  - ~36 dummy matmuls on a memset scratch run during the initial DMA wait
    so the PE_HAM clock gate (1.2 -> 2.4 GHz after ~3.4us of activity) is
    released before the real stream starts.

When the LN affine params and biases are identity (they are for this model
instance; checked at runtime with a general fallback), the final LayerNorm
is also skipped: its input is already a LayerNorm output (per-token mean
exactly 0, variance 1-eps/var), so the final LN is an identity up to
O(eps)~2.5e-6.

HW notes (found the hard way): a ScalarE read of a full 2048B PSUM bank
hard-faults the exec unit -- all ScalarE PSUM reads here are <=1536B.
Activation transposes run as bf16 (2x faster through the PE than f32),
with a bf16 identity matrix as the moving operand.
"""

import os
import sys

import numpy as np

try:
    import concourse.bass as bass
except ImportError:  # grading container should have it on sys.path already
    sys.path.insert(0, "/opt/trn_rl_repo")
    import concourse.bass as bass

import ml_dtypes
import concourse.mybir as mybir
import concourse.tile as tile
from concourse import bacc
from concourse.bass_utils import run_bass_kernel_spmd
from contextlib import ExitStack

F32 = mybir.dt.float32
BF16 = mybir.dt.bfloat16
FP8E4 = mybir.dt.float8e4
AX = mybir.AxisListType
OP = mybir.AluOpType
AF = mybir.ActivationFunctionType
DR = mybir.MatmulPerfMode.DoubleRow
EPS = 1e-5

# Full-size problem config (B=2, S=1024 -> 2048 tokens, 256/core).
# Head: vocab = NQF full 512-chunks + one 256 tail; k-tiles 0..DRKT-1 run as
# fp8 DoubleRow.
CFG_FULL = dict(L=8, H=2048, NC=8, TT=2, V=32000, QV=512, NQF=62, VT=256,
                CH=512, DRKT=8)


def build_nc(cfg, scales, head_scale, triv_ln, fp8_w, use_dr, use_tail):
    L, H, NC, TT = cfg["L"], cfg["H"], cfg["NC"], cfg["TT"]
    V, QV, NQF, VT = cfg["V"], cfg["QV"], cfg["NQF"], cfg["VT"]
    CH, DRKT = cfg["CH"], cfg["DRKT"]
    KT = H // 128
    NCH = H // CH
    DRP = DRKT // 2
    if not use_dr:
        DRKT = DRP = 0
    assert H % CH == 0 and NQF * QV + VT == V
    WDT = FP8E4 if fp8_w else BF16

    nc = bacc.Bacc("TRN2", target_bir_lowering=False, debug=False, num_devices=NC)
    # h0 (layer-0 residual) ships as bf16: halves the startup DMA burst that
    # gates the first layer; costs ~3e-5 relative error on the logits.
    h0 = nc.declare_dram_parameter("h0", [TT, 128, H], BF16, isOutput=False)
    h0T = nc.declare_dram_parameter("h0T", [TT, 128, H], BF16, isOutput=False)
    # weights pre-arranged on host: [L, 128part, KT, H] -> contiguous
    # 8KB-per-partition quarter loads (fast DMA descriptor issue)
    w_ = nc.declare_dram_parameter("w", [L, 128, KT, H], WDT, isOutput=False)
    if not triv_ln:
        lng = nc.declare_dram_parameter("lng", [L, H], BF16, isOutput=False)
        lnb = nc.declare_dram_parameter("lnb", [L, H], BF16, isOutput=False)
        lbias = nc.declare_dram_parameter("lbias", [L, H], BF16, isOutput=False)
        fing = nc.declare_dram_parameter("fing", [H], BF16, isOutput=False)
        finb = nc.declare_dram_parameter("finb", [H], BF16, isOutput=False)
    hw_ = nc.declare_dram_parameter("hw", [NQF, 128, KT, QV], WDT, isOutput=False)
    VTC = VT if use_tail else QV  # tail compute width (QV = padded bisect mode)
    hwt_ = nc.declare_dram_parameter("hwt", [128, KT, VTC], WDT, isOutput=False)
    identb_d = nc.declare_dram_parameter("identb", [128, 128], BF16, isOutput=False)
    eps_d = nc.declare_dram_parameter("eps", [128, 1], F32, isOutput=False)
    # logits leave the core as bf16: halves the output DMA bytes (the head
    # phase is otherwise brushing the HBM ceiling: fp8 weight stream + fp32
    # logits + everything else) at +1e-4 relative error. Host upcasts.
    out = nc.declare_dram_parameter("out", [TT * 128, V], BF16, isOutput=True)

    with tile.TileContext(nc) as tc:
        with ExitStack() as ctx0:
            consts = ctx0.enter_context(tc.tile_pool(name="consts", bufs=1))
            state = ctx0.enter_context(tc.tile_pool(name="state", bufs=4))
            hTp = ctx0.enter_context(tc.tile_pool(name="hT", bufs=2))
            hT8p = ctx0.enter_context(tc.tile_pool(name="hT8", bufs=2))
            wqp = ctx0.enter_context(tc.tile_pool(name="wq", bufs=6))
            outp = ctx0.enter_context(tc.tile_pool(name="outstg", bufs=4))
            smp = ctx0.enter_context(tc.tile_pool(name="small", bufs=16))
            zpool = ctx0.enter_context(tc.tile_pool(name="z", bufs=2))
            wp = ctx0.enter_context(tc.tile_pool(name="w", bufs=12))
            sqp = ctx0.enter_context(tc.tile_pool(name="sq", bufs=2))
            hSp = ctx0.enter_context(tc.tile_pool(name="hS", bufs=2))
            gbp = None
            if not triv_ln:
                gbp = ctx0.enter_context(tc.tile_pool(name="gb", bufs=2))
            psT = ctx0.enter_context(tc.tile_pool(name="psT", bufs=2, space="PSUM"))
            # shared between layer output chunks and head vocab chunks
            psB = ctx0.enter_context(tc.tile_pool(name="psB", bufs=6, space="PSUM"))

            h_cur = []
            hT_cur = []
            for t in range(TT):
                hTt = hTp.tile([128, H], BF16, tag="hT", name=f"hT_p{t}")
                hT_cur.append(hTt)
                st = state.tile([128, H], BF16, name=f"hinit{t}", tag="state")
                h_cur.append(st)
            hT8_cur = [None] * TT
            hnS_pend = [None] * TT  # LN'd bf16 activations awaiting transpose

            def load_wq_sizes(l, sizes, defer=False):
                """Load layer l's weights in slices of sizes[i] k-tiles.
                Returns (tiles, kt_map) with kt_map[kt] = (slice_idx, local).
                defer=True skips the dma_start calls (caller issues them)."""
                tiles, kt_map, k0 = [], [], 0
                for hf, sz in enumerate(sizes):
                    wt = wp.tile([128, sz, H], WDT, tag="w", name=f"w{l}_{hf}")
                    if not defer:
                        nc.sync.dma_start(wt[:], w_[l, :, k0 : k0 + sz, :])
                    tiles.append(wt)
                    for j in range(sz):
                        kt_map.append((hf, j))
                    k0 += sz
                assert k0 == KT
                return tiles, kt_map

            def load_wq(l):
                return load_wq_sizes(l, [2] * 8)

            # DMA issue order tuned for time-to-first-matmul: layer-0 lhsT
            # tile and a single-k-tile first weight slice, then the rest;
            # h0/identb/eps are only needed several us later.
            sizes0 = [1, 1] + [2] * 7
            # split h0T[0]: kt0-3 land first; subtile deps let the first
            # matmuls start before the rest of the tile arrives
            nc.sync.dma_start(hT_cur[0][:, :512], h0T[0, :, :512])
            nc.sync.dma_start(hT_cur[0][:, 512:], h0T[0, :, 512:])
            w_pre = load_wq_sizes(0, sizes0, defer=True)
            w0_tiles = w_pre[0]
            k0s = [sum(sizes0[:i]) for i in range(len(sizes0))]
            for hf in range(1):
                nc.sync.dma_start(
                    w0_tiles[hf][:], w_[0, :, k0s[hf] : k0s[hf] + sizes0[hf], :]
                )
            nc.sync.dma_start(hT_cur[1][:], h0T[1])
            for hf in range(1, len(sizes0)):
                nc.sync.dma_start(
                    w0_tiles[hf][:], w_[0, :, k0s[hf] : k0s[hf] + sizes0[hf], :]
                )
            for t in range(TT):
                nc.sync.dma_start(h_cur[t][:], h0[t])
            identb = consts.tile([128, 128], BF16, name="identb")
            nc.sync.dma_start(identb[:], identb_d[:])
            eps_t = consts.tile([128, 1], F32, name="epst")
            nc.sync.dma_start(eps_t[:], eps_d[:])

            # PE warm-up: the HAM clock gate keeps the PE at 1.2 GHz until
            # ~3.4us of sustained activity. Run dummy transposes on a
            # memset scratch while the first DMAs are in flight so the
            # real matmul stream starts at 2.4 GHz.
            warm_src = consts.tile([128, 128], BF16, name="warmsrc")
            nc.gpsimd.memset(warm_src[:], 0.0)
            warm_ps = psT.tile([128, 128], F32, tag="psTb", name="warmps")
            for i in range(36):
                nc.tensor.matmul(
                    warm_ps[:], lhsT=warm_src[:], rhs=warm_src[:],
                    start=True, stop=True,
                )

            def transpose_bf(src_bf, name):
                """bf16 pre-scaled [128tok, H] -> hT bf16 [128feat-in-
                block, (kt,128tok)]; 1-bank bf16 psum halves, ScalarE
                psum reads kept at 1024B (full-bank reads fault)."""
                dst = hTp.tile([128, H], BF16, tag="hT", name=f"hT{name}")
                for hf in range(2):
                    pT = psT.tile(
                        [128, H // 2], BF16, tag="psTb", name=f"pT{name}_{hf}"
                    )
                    for k in range(KT // 2):
                        kt = hf * (KT // 2) + k
                        nc.tensor.transpose(
                            pT[:, k * 128 : (k + 1) * 128],
                            src_bf[:, kt * 128 : (kt + 1) * 128],
                            identb[:],
                        )
                    lo = hf * (H // 2)
                    if hf == 0:
                        # small first copy: kt0 alone unblocks the next MM
                        # block's first LDWEIGHTS ~250ns earlier
                        nc.scalar.copy(dst[:, lo : lo + 128], pT[:, :128])
                        nc.scalar.copy(dst[:, lo + 128 : lo + 512], pT[:, 128:512])
                    else:
                        nc.scalar.copy(dst[:, lo : lo + 512], pT[:, :512])
                    nc.scalar.copy(dst[:, lo + 512 : lo + 1024], pT[:, 512:])
                return dst

            def transpose_head(src_scaled_bf, src_unscaled_bf, name):
                """Final (head) lhsT: kt 0..DRKT-1 transposed from the
                UNSCALED bf16 activations -> e4m3 (DoubleRow operand);
                kt DRKT..15 from the head_scale-folded bf16 ones."""
                dst = hTp.tile([128, H], BF16, tag="hT", name=f"hT{name}")
                dst8 = None
                if DRKT:
                    dst8 = hT8p.tile(
                        [128, DRKT, 128], FP8E4, tag="hT8", name=f"hT8{name}"
                    )
                    d8v = dst8[:].rearrange("p a b -> p (a b)")
                    for c0 in range(0, DRKT, KT // 2):
                        grp = list(range(c0, min(c0 + KT // 2, DRKT)))
                        pX = psT.tile(
                            [128, H // 2], BF16, tag="psTb", name=f"pX{name}_{c0}"
                        )
                        for j, kt in enumerate(grp):
                            nc.tensor.transpose(
                                pX[:, j * 128 : (j + 1) * 128],
                                src_unscaled_bf[:, kt * 128 : (kt + 1) * 128],
                                identb[:],
                            )
                        n = len(grp) * 128
                        lo = c0 * 128
                        # ScalarE psum reads <=1536B: split if over 768 bf16;
                        # small first copy = first DR pair (kt0,kt1) so the
                        # head's first matmul unblocks early
                        splits = [256, 512, n] if c0 == 0 else [512, n]
                        prev = 0
                        for cut in splits:
                            cut = min(cut, n)
                            if cut > prev:
                                nc.scalar.copy(
                                    d8v[:, lo + prev : lo + cut],
                                    pX[:, prev:cut],
                                )
                            prev = cut
                rem = list(range(DRKT, KT))
                for c0 in range(0, len(rem), KT // 2):
                    grp = rem[c0 : c0 + KT // 2]
                    pT = psT.tile(
                        [128, H // 2], BF16, tag="psTb", name=f"pY{name}_{c0}"
                    )
                    for j, kt in enumerate(grp):
                        nc.tensor.transpose(
                            pT[:, j * 128 : (j + 1) * 128],
                            src_scaled_bf[:, kt * 128 : (kt + 1) * 128],
                            identb[:],
                        )
                    n = len(grp) * 128
                    lo = grp[0] * 128
                    half = min(512, n)
                    nc.scalar.copy(dst[:, lo : lo + half], pT[:, :half])
                    if half < n:
                        nc.scalar.copy(
                            dst[:, lo + half : lo + n], pT[:, half:n]
                        )
                return dst, dst8

            def ln_core(sums, name):
                """Shared LN statistics tail: returns (negmean, rstd)."""
                S = smp.tile([128, 1], F32, tag="s0", name=f"S{name}")
                SS = smp.tile([128, 1], F32, tag="s1", name=f"SS{name}")
                nc.vector.tensor_reduce(
                    S[:], sums[:, 0:NCH], axis=AX.X, op=OP.add
                )
                nc.vector.tensor_reduce(
                    SS[:], sums[:, NCH : 2 * NCH], axis=AX.X, op=OP.add
                )
                negmean = smp.tile([128, 1], F32, tag="s2", name=f"nm{name}")
                nc.vector.tensor_scalar_mul(negmean[:], S[:], -1.0 / H)
                msq = smp.tile([128, 1], F32, tag="s3", name=f"msq{name}")
                nc.vector.tensor_scalar_mul(msq[:], SS[:], 1.0 / H)
                var = smp.tile([128, 1], F32, tag="s4", name=f"var{name}")
                nc.vector.tensor_tensor(var[:], negmean[:], negmean[:], OP.mult)
                nc.vector.tensor_tensor(var[:], msq[:], var[:], OP.subtract)
                std = smp.tile([128, 1], F32, tag="s5", name=f"std{name}")
                nc.scalar.activation(std[:], var[:], AF.Sqrt, bias=eps_t[:])
                rstd = smp.tile([128, 1], F32, tag="s6", name=f"rstd{name}")
                nc.vector.reciprocal(rstd[:], std[:])
                return negmean, rstd

            fg = fb = None
            if not triv_ln:
                fg = gbp.tile([128, H], BF16, tag="g", name="gfin")
                nc.sync.dma_start(fg[:], fing[None, :].to_broadcast((128, H)))
                fb = gbp.tile([128, H], BF16, tag="b", name="bfin")
                nc.sync.dma_start(fb[:], finb[None, :].to_broadcast((128, H)))

            def emit_T(tn, ln):
                """Transpose tile tn's pending LN output into its lhsT for
                stage ln (a layer, or the head when ln == L). Emitted right
                after the MM block that precedes its consumer, and BEFORE
                the current LN's square ops, so its ScalarE drain copies
                aren't queued behind them."""
                if ln < L:
                    hT_cur[tn] = transpose_bf(hnS_pend[tn], f"{ln}_{tn}")
                else:
                    hT_cur[tn], hT8_cur[tn] = transpose_head(
                        hnS_pend[tn], hnS_pend[tn], f"fin{tn}"
                    )

            NWARM = 3  # head chunks prefetched + t0 groups before T_head(t1)
            wq_pre = []  # first head chunks, DMA'd under the last layer
            for l in range(L):
                w_q, kt_map = w_pre if l == 0 else load_wq(l)
                if l == L - 1:
                    for q in range(NWARM):
                        wq = wqp.tile(
                            [128, KT, QV], WDT, tag="wq", name=f"wqpre{q}"
                        )
                        nc.sync.dma_start(wq[:], hw_[q])
                        wq_pre.append(wq)
                g_t = b_t = bias_t = None
                if not triv_ln:
                    g_t = gbp.tile([128, H], BF16, tag="g", name=f"g{l}")
                    nc.sync.dma_start(
                        g_t[:], lng[l][None, :].to_broadcast((128, H))
                    )
                    b_t = gbp.tile([128, H], BF16, tag="b", name=f"b{l}")
                    nc.sync.dma_start(
                        b_t[:], lnb[l][None, :].to_broadcast((128, H))
                    )
                    bias_t = gbp.tile(
                        [128, H], BF16, tag="bias", name=f"bias{l}"
                    )
                    nc.sync.dma_start(
                        bias_t[:], lbias[l][None, :].to_broadcast((128, H))
                    )

                last = l == L - 1
                next_scale = scales[l + 1] if not last else head_scale
                for t in range(TT):
                    hTt = hT_cur[t]
                    ps = [
                        psB.tile([128, CH], F32, tag="psB", name=f"ps{l}_{t}_{i}")
                        for i in range(NCH)
                    ]
                    for half in range(2):
                        for kt in range(KT):
                            hf, loc = kt_map[kt]
                            wt = w_q[hf]
                            for i in (2 * half, 2 * half + 1):
                                nc.tensor.matmul(
                                    ps[i][:],
                                    lhsT=hTt[:, kt * 128 : (kt + 1) * 128],
                                    rhs=wt[:, loc, i * CH : (i + 1) * CH],
                                    start=(kt == 0),
                                    stop=(kt == KT - 1),
                                )
                    # transpose feeding the NEXT PE block, one stage deep:
                    # after MMs(l,t0) -> T for (l,t1); after MMs(l,t1) ->
                    # T for (l+1,t0) (l+1 == L -> the head's t0 operands).
                    if t == 0:
                        if l > 0:
                            emit_T(1, l)
                    elif l + 1 < L or triv_ln:
                        # non-triv: the head operands come from the true
                        # final LN emitted after the loop, not from here
                        emit_T(0, l + 1)
                    z = zpool.tile([128, H], F32, tag="z", name=f"z{l}_{t}")
                    sums = smp.tile(
                        [128, 2 * NCH], F32, tag="sums", name=f"sm{l}_{t}"
                    )
                    resid = h_cur[t]
                    if not triv_ln:
                        hb = zpool.tile(
                            [128, H], F32, tag="hb", name=f"hb{l}_{t}"
                        )
                        nc.vector.tensor_tensor(
                            hb[:], h_cur[t][:], bias_t[:], OP.add
                        )
                        resid = hb
                    # z = psum + resid per chunk, then per-chunk row-sums
                    # (a single fused tensor_tensor_reduce with a PSUM
                    # input faults on HW despite passing CoreSim).
                    for i in range(NCH):
                        nc.vector.tensor_add(
                            z[:, i * CH : (i + 1) * CH],
                            ps[i][:],
                            resid[:, i * CH : (i + 1) * CH],
                        )
                        nc.vector.tensor_reduce(
                            sums[:, i : i + 1],
                            z[:, i * CH : (i + 1) * CH],
                            axis=AX.X,
                            op=OP.add,
                        )
                    for i in range(NCH):
                        sq = sqp.tile(
                            [128, CH], BF16, tag="sq", name=f"sq{l}_{t}_{i}"
                        )
                        nc.scalar.activation(
                            sq[:],
                            z[:, i * CH : (i + 1) * CH],
                            AF.Square,
                            accum_out=sums[:, NCH + i : NCH + i + 1],
                        )
                    negmean, rstd = ln_core(sums, f"{l}_{t}")
                    rstd_s = smp.tile(
                        [128, 1], F32, tag="s7", name=f"rs{l}_{t}"
                    )
                    nc.vector.tensor_scalar_mul(
                        rstd_s[:], rstd[:], float(next_scale)
                    )

                    if triv_ln and last:
                        # hn is mean-0/var-1 by construction, so the
                        # final identity-affine LN is a no-op to O(eps):
                        # emit one UNSCALED bf16 activation feeding both
                        # head operand sets -- no f32 state, no stats.
                        hnS = hSp.tile(
                            [128, H], BF16, tag="hS", name=f"hS{l}_{t}"
                        )
                        nc.vector.tensor_scalar(
                            hnS[:], z[:], negmean[:], rstd[:],
                            OP.add, OP.mult,
                        )
                        hnS_pend[t] = hnS
                        continue

                    hn = state.tile([128, H], F32, tag="state", name=f"h{l}_{t}")
                    hnS = hSp.tile(
                        [128, H], BF16, tag="hS", name=f"hS{l}_{t}"
                    )
                    if triv_ln:
                        # hnS first: it feeds the transpose (latency-
                        # critical); the f32 state is only read next layer.
                        nc.vector.tensor_scalar(
                            hnS[:], z[:], negmean[:], rstd_s[:],
                            OP.add, OP.mult,
                        )
                        nc.vector.tensor_scalar(
                            hn[:], z[:], negmean[:], rstd[:], OP.add, OP.mult
                        )
                    else:
                        nc.vector.tensor_scalar(
                            hn[:], z[:], negmean[:], rstd[:], OP.add, OP.mult
                        )
                        nc.vector.tensor_tensor(hn[:], hn[:], g_t[:], OP.mult)
                        nc.vector.tensor_tensor(hn[:], hn[:], b_t[:], OP.add)
                        nc.vector.tensor_scalar_mul(
                            hnS[:], hn[:], float(next_scale)
                        )
                    h_cur[t] = hn
                    hnS_pend[t] = hnS

            if not triv_ln:
                # general path: true final LN over the f32 state
                for t in range(TT):
                    h8 = h_cur[t]
                    sums = smp.tile(
                        [128, 2 * NCH], F32, tag="sums", name=f"smf{t}"
                    )
                    nc.vector.tensor_reduce(
                        sums[:, 0:1], h8[:], axis=AX.X, op=OP.add
                    )
                    # only sums[:,0] used for S in this path
                    for i in range(1, NCH):
                        nc.vector.tensor_scalar_mul(
                            sums[:, i : i + 1], sums[:, 0:1], 0.0
                        )
                    for i in range(NCH):
                        sq = sqp.tile(
                            [128, CH], BF16, tag="sq", name=f"sqf{t}_{i}"
                        )
                        nc.scalar.activation(
                            sq[:],
                            h8[:, i * CH : (i + 1) * CH],
                            AF.Square,
                            accum_out=sums[:, NCH + i : NCH + i + 1],
                        )
                    negmean, rstd = ln_core(sums, f"fin{t}")
                    hfin = state.tile(
                        [128, H], F32, tag="state", name=f"hf{t}"
                    )
                    nc.vector.tensor_scalar(
                        hfin[:], h8[:], negmean[:], rstd[:], OP.add, OP.mult
                    )
                    nc.vector.tensor_tensor(hfin[:], hfin[:], fg[:], OP.mult)
                    nc.vector.tensor_tensor(hfin[:], hfin[:], fb[:], OP.add)
                    hnS = hSp.tile([128, H], BF16, tag="hS", name=f"hSf{t}")
                    nc.vector.tensor_scalar_mul(hnS[:], hfin[:], 1.0)
                    hnS_pend[t] = hnS
                if not triv_ln:
                    emit_T(0, L)

            # ---- head: own 256 tokens x full vocab, streamed fp8 weights ----
            def head_group(q, t, wq, cols, out_cols=None):
                out_cols = cols if out_cols is None else out_cols
                # single accumulation group: DR (e4m3) k-tiles then bf16
                # k-tiles into one bank; both lhsT operands are UNSCALED,
                # the ternary head scale is applied once at drain.
                pb = psB.tile([128, cols], F32, tag="psB", name=f"pb{q}_{t}")
                for j in range(DRP):
                    nc.tensor.matmul(
                        pb[:],
                        lhsT=hT8_cur[t][:, 2 * j : 2 * j + 2, :],
                        rhs=wq[:, 2 * j : 2 * j + 2, :],
                        start=(j == 0),
                        stop=False,
                        perf_mode=DR,
                        skip_group_check=True,
                    )
                for kt in range(DRKT, KT):
                    nc.tensor.matmul(
                        pb[:],
                        lhsT=hT_cur[t][:, kt * 128 : (kt + 1) * 128],
                        rhs=wq[:, kt, :],
                        start=(DRP == 0 and kt == DRKT),
                        stop=(kt == KT - 1),
                        skip_group_check=True,
                    )
                # ScalarE PSUM reads must stay under one full 2048B bank
                o_t = outp.tile([128, cols], BF16, tag="ostg", name=f"o{q}_{t}")
                for c0 in range(0, cols, 256):
                    nc.scalar.activation(
                        o_t[:, c0 : c0 + 256], pb[:, c0 : c0 + 256],
                        AF.Copy, scale=float(head_scale),
                    )
                nc.sync.dma_start(
                    out[t * 128 : (t + 1) * 128, q * QV : q * QV + out_cols],
                    o_t[:, :out_cols],
                )

            # transition: T_head(t0) was emitted after the last layer's t1
            # matmuls; run q=0..NWARM-1 for t0 (covers t1's LN chain), then
            # T_head(t1), its warm groups, then the steady loop.
            for q in range(NWARM):
                head_group(q, 0, wq_pre[q], QV)
            emit_T(1, L)
            for q in range(NWARM):
                head_group(q, 1, wq_pre[q], QV)
            for q in range(NWARM, NQF):
                wq = wqp.tile([128, KT, QV], WDT, tag="wq", name=f"wq{q}")
                nc.sync.dma_start(wq[:], hw_[q])
                for t in range(TT):
                    head_group(q, t, wq, QV)
            # 256-wide vocab tail (32000 = 62*512 + 256)
            wqt = wqp.tile([128, KT, VTC], WDT, tag="wq", name="wqtail")
            nc.sync.dma_start(wqt[:], hwt_[:])
            for t in range(TT):
                head_group(NQF, t, wqt, VTC, out_cols=VT)

    return nc


def _ternary(wmat):
    """Exact {-1,0,1} ternary tensor + fp32 scale, matching the reference."""
    w = np.asarray(wmat, dtype=np.float32)
    s = np.mean(np.abs(w), dtype=np.float32)
    t = np.clip(np.rint(w / (s + np.float32(1e-8))), -1.0, 1.0).astype(np.float32)
    return t, float(s)


_NC_CACHE = {}
_LAST_RESULTS = None


def kernel(**inputs):
    global _LAST_RESULTS
    cfg = CFG_FULL
    L, H, NC, TT, V, QV, NQF, VT = (
        cfg["L"], cfg["H"], cfg["NC"], cfg["TT"], cfg["V"], cfg["QV"],
        cfg["NQF"], cfg["VT"],
    )
    KT = H // 128
    TPC = TT * 128  # tokens per core
    BF = ml_dtypes.bfloat16
    F8 = ml_dtypes.float8_e4m3fn
    fp8_w = not bool(int(os.environ.get("TRIKERNEL_BF16_W", "0")))
    use_dr = fp8_w and not bool(int(os.environ.get("TRIKERNEL_NO_DR", "0")))
    use_tail = bool(int(os.environ.get("TRIKERNEL_TAIL", "1")))
    WNP = F8 if fp8_w else BF

    ids = np.asarray(inputs["input_ids"]).astype(np.int64).reshape(-1)
    embed = np.asarray(inputs["embed"], dtype=np.float32)
    layer_w = np.asarray(inputs["layer_w"], dtype=np.float32)
    layer_b = np.asarray(inputs["layer_b"], dtype=np.float32)
    ln_g = np.asarray(inputs["ln_g"], dtype=np.float32)
    ln_b = np.asarray(inputs["ln_b"], dtype=np.float32)
    final_g = np.asarray(inputs["final_g"], dtype=np.float32)
    final_b = np.asarray(inputs["final_b"], dtype=np.float32)
    head_w = np.asarray(inputs["head_w"], dtype=np.float32)

    # trivial-affine specialization: the LN scale/shift and layer bias are
    # identity in this model instance; skip them on-chip when so.
    triv_ln = bool(
        np.all(ln_g == 1.0) and np.all(ln_b == 0.0) and np.all(layer_b == 0.0)
        and np.all(final_g == 1.0) and np.all(final_b == 0.0)
    )

    h0_full = embed[ids]  # [NTOK, H] fp32

    scales = []
    wT = np.empty([L, 128, KT, H], dtype=WNP)
    for l in range(L):
        t, s = _ternary(layer_w[l])
        scales.append(s)
        # [H(o), H(k)] -> transpose -> [KT,128,H] -> partition-major
        wT[l] = np.ascontiguousarray(
            t.T.reshape(KT, 128, H).transpose(1, 0, 2)
        ).astype(WNP)
    th, head_scale = _ternary(head_w)
    # head weights laid out so each [128, KT, QV] chunk is a single
    # contiguous 8KB-per-partition DMA: hw8[q, p, kt, v]; the 256-wide
    # vocab tail is its own tensor.
    thT = th.T  # [H, V]
    hw8 = np.ascontiguousarray(
        thT[:, : NQF * QV].reshape(KT, 128, NQF, QV).transpose(2, 1, 0, 3)
    ).astype(WNP)
    VTC = VT if use_tail else QV
    thT_tail = np.zeros((H, VTC), dtype=np.float32)
    thT_tail[:, :VT] = thT[:, NQF * QV :]
    hwt = np.ascontiguousarray(
        thT_tail.reshape(KT, 128, VTC).transpose(1, 0, 2)
    ).astype(WNP)

    key = (tuple(sorted(cfg.items())), tuple(scales), head_scale, triv_ln,
           fp8_w, use_dr, use_tail)
    if key not in _NC_CACHE:
        _NC_CACHE.clear()
        nc = build_nc(cfg, scales, head_scale, triv_ln, fp8_w, use_dr,
                      use_tail)
        # Bacc.finalize runs the TRN2 legalization passes (1-wait-per-
        # instruction event-semaphore split, matmul->ldweights wait motion,
        # register allocation). The PJRT exec path serializes nc as-is.
        nc.finalize()
        _NC_CACHE[key] = nc
    nc = _NC_CACHE[key]

    common = {
        "w": wT,
        "hw": hw8,
        "hwt": hwt,
        "identb": np.eye(128, dtype=BF),
        "eps": np.full((128, 1), EPS, np.float32),
    }
    if not triv_ln:
        common.update(
            lng=ln_g.astype(BF),
            lnb=ln_b.astype(BF),
            lbias=layer_b.astype(BF),
            fing=final_g.astype(BF),
            finb=final_b.astype(BF),
        )
    in_maps = []
    for c in range(NC):
        h0c = np.ascontiguousarray(
            h0_full[c * TPC : (c + 1) * TPC].reshape(TT, 128, H)
        )
        # host-side pre-transpose of the layer-0 lhsT (scaled, bf16)
        h0Tc = np.ascontiguousarray(
            (h0c.reshape(TT, 128, KT, 128).transpose(0, 3, 2, 1)
             * np.float32(scales[0])).reshape(TT, 128, H)
        ).astype(BF)
        in_maps.append(dict(common, h0=h0c.astype(BF), h0T=h0Tc))

    trace = bool(int(os.environ.get("TRIKERNEL_TRACE", "0")))
    res = run_bass_kernel_spmd(nc, in_maps, core_ids=list(range(NC)), trace=trace)
    _LAST_RESULTS = res

    full = np.concatenate(
        [np.asarray(res.results[c]["out"]).astype(np.float32) for c in range(NC)],
        axis=0,
    )  # [NTOK, V]
    return full.reshape(2, 1024, 32000)


# revision 43
# speedup vs baseline: 1.0050x; 1.0050x over previous
"""Trainium2 Bass kernel: 8-layer ternary (BitNet-1.58) dense transformer.

Model (per reference):
    h = embed[input_ids]                                  # (B=2, S=1024, H=2048)
    8x: y = h @ ternary(W_l)^T + b_l ; h = LN(y + h)*g+b  # H=2048
    h = LN(h)*final_g + final_b
    logits = h @ ternary(head_W)^T                        # (B, S, V=32000)

Sharding over 8 NeuronCores (fully local, no collectives):
  - Layers: data-parallel over the 2048 tokens (256 tokens/core). Each core
    streams the full ternary layer weights as exact {-1,0,+1} fp8(e4m3).
  - Head: ALSO data-parallel over tokens: each core computes its own 256
    tokens x the full 32000-entry vocab, streaming fp8 head weights
    chunk-by-chunk, overlapped with compute. No collectives at all.

Head matmul runs mixed precision: k-tiles 0..DRKT-1 via fp8 DoubleRow (2
k-tiles per instruction, activations rounded to e4m3), the rest as bf16
activations x fp8 weights at full precision. DRKT=8 costs ~1.91e-2 relative
error on the logits (vs the 2e-2 budget; host-simulated AND confirmed on HW)
and saves 2 of 13 matmul slots per vocab chunk vs DRKT=6.

The vocab is NOT padded on the compute side: 62 chunks of 512 plus one tail
chunk of 256 (32000 = 62*512 + 256), saving the 256 dead columns.

Schedule notes (from perfetto/NTFF analysis of the 640us baseline):
  - The PE stream is otherwise dense; the bubbles were (a) per-layer-tile
    ~0.8us waits of the activation transposes on the LayerNorm vector chain
    and (b) a 6.2us layers->head transition stall. Both are fixed by
    software-pipelining the transposes one stage deeper: the transpose
    feeding PE block N+1 is emitted right after block N's matmuls (and
    before the current LN's ScalarE squares, so its drain copies aren't
    queued behind them); ~14us of matmuls cover the ~6us LN chain latency.
    The first drain copy is kt0-only so the next block's first LDWEIGHTS
    unblocks ~250ns earlier.
  - Head transition: T_head(t0) is emitted inside the last layer;
    then q=0..NWARM-1 groups for t0 (covering t1's final LN chain),
    T_head(t1), its warm groups, then the steady loop. Head weight DMA for
    the warm chunks is issued before the last layer's matmuls.
  - The LN row-sum runs per 512-chunk right behind each z = psum + resid
    chunk (replacing one 2.3us full-row reduce). NB: DVE
    InstTensorTensorReduce faults on HW via this toolchain (both with PSUM
    and SBUF inputs) despite passing CoreSim -- do not use it.
  - bf16 logits + bf16 h0 keep the head phase under the per-core DMA
    ceiling (fp8 weight stream at ~200 GB/s/core + output writes); with
    fp32 logits the weight stream starves and the PE idles ~1.5us every
    few vocab chunks.
  - ~36 dummy matmuls on a memset scratch run during the initial DMA wait
    so the PE_HAM clock gate (1.2 -> 2.4 GHz after ~3.4us of activity) is
    released before the real stream starts.

When the LN affine params and biases are identity (they are for this model
instance; checked at runtime with a general fallback), the final LayerNorm
is also skipped: its input is already a LayerNorm output (per-token mean
exactly 0, variance 1-eps/var), so the final LN is an identity up to
O(eps)~2.5e-6.

HW notes (found the hard way): a ScalarE read of a full 2048B PSUM bank
hard-faults the exec unit -- all ScalarE PSUM reads here are <=1536B.
Activation transposes run as bf16 (2x faster through the PE than f32),
with a bf16 identity matrix as the moving operand.
"""

import os
import sys

import numpy as np

try:
    import concourse.bass as bass
except ImportError:  # grading container should have it on sys.path already
    sys.path.insert(0, "/opt/trn_rl_repo")
    import concourse.bass as bass

import ml_dtypes
import concourse.mybir as mybir
import concourse.tile as tile
from concourse import bacc
from concourse.bass_utils import run_bass_kernel_spmd
from contextlib import ExitStack

F32 = mybir.dt.float32
BF16 = mybir.dt.bfloat16
FP8E4 = mybir.dt.float8e4
AX = mybir.AxisListType
OP = mybir.AluOpType
AF = mybir.ActivationFunctionType
DR = mybir.MatmulPerfMode.DoubleRow
EPS = 1e-5

# Full-size problem config (B=2, S=1024 -> 2048 tokens, 256/core).
# Head: vocab = NQF full 512-chunks + one 256 tail; k-tiles 0..DRKT-1 run as
# fp8 DoubleRow.
CFG_FULL = dict(L=8, H=2048, NC=8, TT=2, V=32000, QV=512, NQF=62, VT=256,
                CH=512, DRKT=8)


def build_nc(cfg, scales, head_scale, triv_ln, fp8_w, use_dr, use_tail):
    L, H, NC, TT = cfg["L"], cfg["H"], cfg["NC"], cfg["TT"]
    V, QV, NQF, VT = cfg["V"], cfg["QV"], cfg["NQF"], cfg["VT"]
    CH, DRKT = cfg["CH"], cfg["DRKT"]
    KT = H // 128
    NCH = H // CH
    DRP = DRKT // 2
    if not use_dr:
        DRKT = DRP = 0
    assert H % CH == 0 and NQF * QV + VT == V
    WDT = FP8E4 if fp8_w else BF16

    nc = bacc.Bacc("TRN2", target_bir_lowering=False, debug=False, num_devices=NC)
    # h0 (layer-0 residual) ships as bf16: halves the startup DMA burst that
    # gates the first layer; costs ~3e-5 relative error on the logits.
    h0 = nc.declare_dram_parameter("h0", [TT, 128, H], BF16, isOutput=False)
    h0T = nc.declare_dram_parameter("h0T", [TT, 128, H], BF16, isOutput=False)
    # weights pre-arranged on host: [L, 128part, KT, H] -> contiguous
    # 8KB-per-partition quarter loads (fast DMA descriptor issue)
    w_ = nc.declare_dram_parameter("w", [L, 128, KT, H], WDT, isOutput=False)
    if not triv_ln:
        lng = nc.declare_dram_parameter("lng", [L, H], BF16, isOutput=False)
        lnb = nc.declare_dram_parameter("lnb", [L, H], BF16, isOutput=False)
        lbias = nc.declare_dram_parameter("lbias", [L, H], BF16, isOutput=False)
        fing = nc.declare_dram_parameter("fing", [H], BF16, isOutput=False)
        finb = nc.declare_dram_parameter("finb", [H], BF16, isOutput=False)
    hw_ = nc.declare_dram_parameter("hw", [NQF, 128, KT, QV], WDT, isOutput=False)
    VTC = VT if use_tail else QV  # tail compute width (QV = padded bisect mode)
    hwt_ = nc.declare_dram_parameter("hwt", [128, KT, VTC], WDT, isOutput=False)
    identb_d = nc.declare_dram_parameter("identb", [128, 128], BF16, isOutput=False)
    eps_d = nc.declare_dram_parameter("eps", [128, 1], F32, isOutput=False)
    # logits leave the core as bf16: halves the output DMA bytes (the head
    # phase is otherwise brushing the HBM ceiling: fp8 weight stream + fp32
    # logits + everything else) at +1e-4 relative error. Host upcasts.
    out = nc.declare_dram_parameter("out", [TT * 128, V], BF16, isOutput=True)

    with tile.TileContext(nc) as tc:
        with ExitStack() as ctx0:
            consts = ctx0.enter_context(tc.tile_pool(name="consts", bufs=1))
            state = ctx0.enter_context(tc.tile_pool(name="state", bufs=4))
            hTp = ctx0.enter_context(tc.tile_pool(name="hT", bufs=2))
            hT8p = ctx0.enter_context(tc.tile_pool(name="hT8", bufs=2))
            wqp = ctx0.enter_context(tc.tile_pool(name="wq", bufs=6))
            outp = ctx0.enter_context(tc.tile_pool(name="outstg", bufs=4))
            smp = ctx0.enter_context(tc.tile_pool(name="small", bufs=16))
            zpool = ctx0.enter_context(tc.tile_pool(name="z", bufs=2))
            wp = ctx0.enter_context(tc.tile_pool(name="w", bufs=12))
            sqp = ctx0.enter_context(tc.tile_pool(name="sq", bufs=2))
            hSp = ctx0.enter_context(tc.tile_pool(name="hS", bufs=2))
            gbp = None
            if not triv_ln:
                gbp = ctx0.enter_context(tc.tile_pool(name="gb", bufs=2))
            psT = ctx0.enter_context(tc.tile_pool(name="psT", bufs=2, space="PSUM"))
            # shared between layer output chunks and head vocab chunks
            psB = ctx0.enter_context(tc.tile_pool(name="psB", bufs=6, space="PSUM"))

            h_cur = []
            hT_cur = []
            for t in range(TT):
                hTt = hTp.tile([128, H], BF16, tag="hT", name=f"hT_p{t}")
                hT_cur.append(hTt)
                st = state.tile([128, H], BF16, name=f"hinit{t}", tag="state")
                h_cur.append(st)
            hT8_cur = [None] * TT
            hnS_pend = [None] * TT  # LN'd bf16 activations awaiting transpose

            def load_wq_sizes(l, sizes, defer=False):
                """Load layer l's weights in slices of sizes[i] k-tiles.
                Returns (tiles, kt_map) with kt_map[kt] = (slice_idx, local).
                defer=True skips the dma_start calls (caller issues them)."""
                tiles, kt_map, k0 = [], [], 0
                for hf, sz in enumerate(sizes):
                    wt = wp.tile([128, sz, H], WDT, tag="w", name=f"w{l}_{hf}")
                    if not defer:
                        nc.sync.dma_start(wt[:], w_[l, :, k0 : k0 + sz, :])
                    tiles.append(wt)
                    for j in range(sz):
                        kt_map.append((hf, j))
                    k0 += sz
                assert k0 == KT
                return tiles, kt_map

            def load_wq(l):
                return load_wq_sizes(l, [2] * 8)

            # DMA issue order tuned for time-to-first-matmul: layer-0 lhsT
            # tile and a single-k-tile first weight slice, then the rest;
            # h0/identb/eps are only needed several us later.
            sizes0 = [1, 1] + [2] * 7
            nc.sync.dma_start(hT_cur[0][:], h0T[0])
            w_pre = load_wq_sizes(0, sizes0, defer=True)
            w0_tiles = w_pre[0]
            k0s = [sum(sizes0[:i]) for i in range(len(sizes0))]
            for hf in range(1):
                nc.sync.dma_start(
                    w0_tiles[hf][:], w_[0, :, k0s[hf] : k0s[hf] + sizes0[hf], :]
                )
            nc.sync.dma_start(hT_cur[1][:], h0T[1])
            for hf in range(1, len(sizes0)):
                nc.sync.dma_start(
                    w0_tiles[hf][:], w_[0, :, k0s[hf] : k0s[hf] + sizes0[hf], :]
                )
            for t in range(TT):
                nc.sync.dma_start(h_cur[t][:], h0[t])
            identb = consts.tile([128, 128], BF16, name="identb")
            nc.sync.dma_start(identb[:], identb_d[:])
            eps_t = consts.tile([128, 1], F32, name="epst")
            nc.sync.dma_start(eps_t[:], eps_d[:])

            # PE warm-up: the HAM clock gate keeps the PE at 1.2 GHz until
            # ~3.4us of sustained activity. Run dummy transposes on a
            # memset scratch while the first DMAs are in flight so the
            # real matmul stream starts at 2.4 GHz.
            warm_src = consts.tile([128, 128], BF16, name="warmsrc")
            nc.gpsimd.memset(warm_src[:], 0.0)
            warm_ps = psT.tile([128, 128], F32, tag="psTb", name="warmps")
            for i in range(36):
                nc.tensor.matmul(
                    warm_ps[:], lhsT=warm_src[:], rhs=warm_src[:],
                    start=True, stop=True,
                )

            def transpose_bf(src_bf, name):
                """bf16 pre-scaled [128tok, H] -> hT bf16 [128feat-in-
                block, (kt,128tok)]; 1-bank bf16 psum halves, ScalarE
                psum reads kept at 1024B (full-bank reads fault)."""
                dst = hTp.tile([128, H], BF16, tag="hT", name=f"hT{name}")
                for hf in range(2):
                    pT = psT.tile(
                        [128, H // 2], BF16, tag="psTb", name=f"pT{name}_{hf}"
                    )
                    for k in range(KT // 2):
                        kt = hf * (KT // 2) + k
                        nc.tensor.transpose(
                            pT[:, k * 128 : (k + 1) * 128],
                            src_bf[:, kt * 128 : (kt + 1) * 128],
                            identb[:],
                        )
                    lo = hf * (H // 2)
                    if hf == 0:
                        # small first copy: kt0 alone unblocks the next MM
                        # block's first LDWEIGHTS ~250ns earlier
                        nc.scalar.copy(dst[:, lo : lo + 128], pT[:, :128])
                        nc.scalar.copy(dst[:, lo + 128 : lo + 512], pT[:, 128:512])
                    else:
                        nc.scalar.copy(dst[:, lo : lo + 512], pT[:, :512])
                    nc.scalar.copy(dst[:, lo + 512 : lo + 1024], pT[:, 512:])
                return dst

            def transpose_head(src_scaled_bf, src_unscaled_bf, name):
                """Final (head) lhsT: kt 0..DRKT-1 transposed from the
                UNSCALED bf16 activations -> e4m3 (DoubleRow operand);
                kt DRKT..15 from the head_scale-folded bf16 ones."""
                dst = hTp.tile([128, H], BF16, tag="hT", name=f"hT{name}")
                dst8 = None
                if DRKT:
                    dst8 = hT8p.tile(
                        [128, DRKT, 128], FP8E4, tag="hT8", name=f"hT8{name}"
                    )
                    d8v = dst8[:].rearrange("p a b -> p (a b)")
                    for c0 in range(0, DRKT, KT // 2):
                        grp = list(range(c0, min(c0 + KT // 2, DRKT)))
                        pX = psT.tile(
                            [128, H // 2], BF16, tag="psTb", name=f"pX{name}_{c0}"
                        )
                        for j, kt in enumerate(grp):
                            nc.tensor.transpose(
                                pX[:, j * 128 : (j + 1) * 128],
                                src_unscaled_bf[:, kt * 128 : (kt + 1) * 128],
                                identb[:],
                            )
                        n = len(grp) * 128
                        lo = c0 * 128
                        # ScalarE psum reads <=1536B: split if over 768 bf16;
                        # small first copy = first DR pair (kt0,kt1) so the
                        # head's first matmul unblocks early
                        splits = [256, 512, n] if c0 == 0 else [512, n]
                        prev = 0
                        for cut in splits:
                            cut = min(cut, n)
                            if cut > prev:
                                nc.scalar.copy(
                                    d8v[:, lo + prev : lo + cut],
                                    pX[:, prev:cut],
                                )
                            prev = cut
                rem = list(range(DRKT, KT))
                for c0 in range(0, len(rem), KT // 2):
                    grp = rem[c0 : c0 + KT // 2]
                    pT = psT.tile(
                        [128, H // 2], BF16, tag="psTb", name=f"pY{name}_{c0}"
                    )
                    for j, kt in enumerate(grp):
                        nc.tensor.transpose(
                            pT[:, j * 128 : (j + 1) * 128],
                            src_scaled_bf[:, kt * 128 : (kt + 1) * 128],
                            identb[:],
                        )
                    n = len(grp) * 128
                    lo = grp[0] * 128
                    half = min(512, n)
                    nc.scalar.copy(dst[:, lo : lo + half], pT[:, :half])
                    if half < n:
                        nc.scalar.copy(
                            dst[:, lo + half : lo + n], pT[:, half:n]
                        )
                return dst, dst8

            def ln_core(sums, name):
                """Shared LN statistics tail: returns (negmean, rstd)."""
                S = smp.tile([128, 1], F32, tag="s0", name=f"S{name}")
                SS = smp.tile([128, 1], F32, tag="s1", name=f"SS{name}")
                nc.vector.tensor_reduce(
                    S[:], sums[:, 0:NCH], axis=AX.X, op=OP.add
                )
                nc.vector.tensor_reduce(
                    SS[:], sums[:, NCH : 2 * NCH], axis=AX.X, op=OP.add
                )
                negmean = smp.tile([128, 1], F32, tag="s2", name=f"nm{name}")
                nc.vector.tensor_scalar_mul(negmean[:], S[:], -1.0 / H)
                msq = smp.tile([128, 1], F32, tag="s3", name=f"msq{name}")
                nc.vector.tensor_scalar_mul(msq[:], SS[:], 1.0 / H)
                var = smp.tile([128, 1], F32, tag="s4", name=f"var{name}")
                nc.vector.tensor_tensor(var[:], negmean[:], negmean[:], OP.mult)
                nc.vector.tensor_tensor(var[:], msq[:], var[:], OP.subtract)
                std = smp.tile([128, 1], F32, tag="s5", name=f"std{name}")
                nc.scalar.activation(std[:], var[:], AF.Sqrt, bias=eps_t[:])
                rstd = smp.tile([128, 1], F32, tag="s6", name=f"rstd{name}")
                nc.vector.reciprocal(rstd[:], std[:])
                return negmean, rstd

            fg = fb = None
            if not triv_ln:
                fg = gbp.tile([128, H], BF16, tag="g", name="gfin")
                nc.sync.dma_start(fg[:], fing[None, :].to_broadcast((128, H)))
                fb = gbp.tile([128, H], BF16, tag="b", name="bfin")
                nc.sync.dma_start(fb[:], finb[None, :].to_broadcast((128, H)))

            def emit_T(tn, ln):
                """Transpose tile tn's pending LN output into its lhsT for
                stage ln (a layer, or the head when ln == L). Emitted right
                after the MM block that precedes its consumer, and BEFORE
                the current LN's square ops, so its ScalarE drain copies
                aren't queued behind them."""
                if ln < L:
                    hT_cur[tn] = transpose_bf(hnS_pend[tn], f"{ln}_{tn}")
                else:
                    hT_cur[tn], hT8_cur[tn] = transpose_head(
                        hnS_pend[tn], hnS_pend[tn], f"fin{tn}"
                    )

            NWARM = 3  # head chunks prefetched + t0 groups before T_head(t1)
            wq_pre = []  # first head chunks, DMA'd under the last layer
            for l in range(L):
                w_q, kt_map = w_pre if l == 0 else load_wq(l)
                if l == L - 1:
                    for q in range(NWARM):
                        wq = wqp.tile(
                            [128, KT, QV], WDT, tag="wq", name=f"wqpre{q}"
                        )
                        nc.sync.dma_start(wq[:], hw_[q])
                        wq_pre.append(wq)
                g_t = b_t = bias_t = None
                if not triv_ln:
                    g_t = gbp.tile([128, H], BF16, tag="g", name=f"g{l}")
                    nc.sync.dma_start(
                        g_t[:], lng[l][None, :].to_broadcast((128, H))
                    )
                    b_t = gbp.tile([128, H], BF16, tag="b", name=f"b{l}")
                    nc.sync.dma_start(
                        b_t[:], lnb[l][None, :].to_broadcast((128, H))
                    )
                    bias_t = gbp.tile(
                        [128, H], BF16, tag="bias", name=f"bias{l}"
                    )
                    nc.sync.dma_start(
                        bias_t[:], lbias[l][None, :].to_broadcast((128, H))
                    )

                last = l == L - 1
                next_scale = scales[l + 1] if not last else head_scale
                for t in range(TT):
                    hTt = hT_cur[t]
                    ps = [
                        psB.tile([128, CH], F32, tag="psB", name=f"ps{l}_{t}_{i}")
                        for i in range(NCH)
                    ]
                    for half in range(2):
                        for kt in range(KT):
                            hf, loc = kt_map[kt]
                            wt = w_q[hf]
                            for i in (2 * half, 2 * half + 1):
                                nc.tensor.matmul(
                                    ps[i][:],
                                    lhsT=hTt[:, kt * 128 : (kt + 1) * 128],
                                    rhs=wt[:, loc, i * CH : (i + 1) * CH],
                                    start=(kt == 0),
                                    stop=(kt == KT - 1),
                                )
                    # transpose feeding the NEXT PE block, one stage deep:
                    # after MMs(l,t0) -> T for (l,t1); after MMs(l,t1) ->
                    # T for (l+1,t0) (l+1 == L -> the head's t0 operands).
                    if t == 0:
                        if l > 0:
                            emit_T(1, l)
                    elif l + 1 < L or triv_ln:
                        # non-triv: the head operands come from the true
                        # final LN emitted after the loop, not from here
                        emit_T(0, l + 1)
                    z = zpool.tile([128, H], F32, tag="z", name=f"z{l}_{t}")
                    sums = smp.tile(
                        [128, 2 * NCH], F32, tag="sums", name=f"sm{l}_{t}"
                    )
                    resid = h_cur[t]
                    if not triv_ln:
                        hb = zpool.tile(
                            [128, H], F32, tag="hb", name=f"hb{l}_{t}"
                        )
                        nc.vector.tensor_tensor(
                            hb[:], h_cur[t][:], bias_t[:], OP.add
                        )
                        resid = hb
                    # z = psum + resid per chunk, then per-chunk row-sums
                    # (a single fused tensor_tensor_reduce with a PSUM
                    # input faults on HW despite passing CoreSim).
                    for i in range(NCH):
                        nc.vector.tensor_add(
                            z[:, i * CH : (i + 1) * CH],
                            ps[i][:],
                            resid[:, i * CH : (i + 1) * CH],
                        )
                        nc.vector.tensor_reduce(
                            sums[:, i : i + 1],
                            z[:, i * CH : (i + 1) * CH],
                            axis=AX.X,
                            op=OP.add,
                        )
                    for i in range(NCH):
                        sq = sqp.tile(
                            [128, CH], BF16, tag="sq", name=f"sq{l}_{t}_{i}"
                        )
                        nc.scalar.activation(
                            sq[:],
                            z[:, i * CH : (i + 1) * CH],
                            AF.Square,
                            accum_out=sums[:, NCH + i : NCH + i + 1],
                        )
                    negmean, rstd = ln_core(sums, f"{l}_{t}")
                    rstd_s = smp.tile(
                        [128, 1], F32, tag="s7", name=f"rs{l}_{t}"
                    )
                    nc.vector.tensor_scalar_mul(
                        rstd_s[:], rstd[:], float(next_scale)
                    )

                    if triv_ln and last:
                        # hn is mean-0/var-1 by construction, so the
                        # final identity-affine LN is a no-op to O(eps):
                        # emit one UNSCALED bf16 activation feeding both
                        # head operand sets -- no f32 state, no stats.
                        hnS = hSp.tile(
                            [128, H], BF16, tag="hS", name=f"hS{l}_{t}"
                        )
                        nc.vector.tensor_scalar(
                            hnS[:], z[:], negmean[:], rstd[:],
                            OP.add, OP.mult,
                        )
                        hnS_pend[t] = hnS
                        continue

                    hn = state.tile([128, H], F32, tag="state", name=f"h{l}_{t}")
                    hnS = hSp.tile(
                        [128, H], BF16, tag="hS", name=f"hS{l}_{t}"
                    )
                    if triv_ln:
                        # hnS first: it feeds the transpose (latency-
                        # critical); the f32 state is only read next layer.
                        nc.vector.tensor_scalar(
                            hnS[:], z[:], negmean[:], rstd_s[:],
                            OP.add, OP.mult,
                        )
                        nc.vector.tensor_scalar(
                            hn[:], z[:], negmean[:], rstd[:], OP.add, OP.mult
                        )
                    else:
                        nc.vector.tensor_scalar(
                            hn[:], z[:], negmean[:], rstd[:], OP.add, OP.mult
                        )
                        nc.vector.tensor_tensor(hn[:], hn[:], g_t[:], OP.mult)
                        nc.vector.tensor_tensor(hn[:], hn[:], b_t[:], OP.add)
                        nc.vector.tensor_scalar_mul(
                            hnS[:], hn[:], float(next_scale)
                        )
                    h_cur[t] = hn
                    hnS_pend[t] = hnS

            if not triv_ln:
                # general path: true final LN over the f32 state
                for t in range(TT):
                    h8 = h_cur[t]
                    sums = smp.tile(
                        [128, 2 * NCH], F32, tag="sums", name=f"smf{t}"
                    )
                    nc.vector.tensor_reduce(
                        sums[:, 0:1], h8[:], axis=AX.X, op=OP.add
                    )
                    # only sums[:,0] used for S in this path
                    for i in range(1, NCH):
                        nc.vector.tensor_scalar_mul(
                            sums[:, i : i + 1], sums[:, 0:1], 0.0
                        )
                    for i in range(NCH):
                        sq = sqp.tile(
                            [128, CH], BF16, tag="sq", name=f"sqf{t}_{i}"
                        )
                        nc.scalar.activation(
                            sq[:],
                            h8[:, i * CH : (i + 1) * CH],
                            AF.Square,
                            accum_out=sums[:, NCH + i : NCH + i + 1],
                        )
                    negmean, rstd = ln_core(sums, f"fin{t}")
                    hfin = state.tile(
                        [128, H], F32, tag="state", name=f"hf{t}"
                    )
                    nc.vector.tensor_scalar(
                        hfin[:], h8[:], negmean[:], rstd[:], OP.add, OP.mult
                    )
                    nc.vector.tensor_tensor(hfin[:], hfin[:], fg[:], OP.mult)
                    nc.vector.tensor_tensor(hfin[:], hfin[:], fb[:], OP.add)
                    hnS = hSp.tile([128, H], BF16, tag="hS", name=f"hSf{t}")
                    nc.vector.tensor_scalar_mul(hnS[:], hfin[:], 1.0)
                    hnS_pend[t] = hnS
                if not triv_ln:
                    emit_T(0, L)

            # ---- head: own 256 tokens x full vocab, streamed fp8 weights ----
            def head_group(q, t, wq, cols, out_cols=None):
                out_cols = cols if out_cols is None else out_cols
                # single accumulation group: DR (e4m3) k-tiles then bf16
                # k-tiles into one bank; both lhsT operands are UNSCALED,
                # the ternary head scale is applied once at drain.
                pb = psB.tile([128, cols], F32, tag="psB", name=f"pb{q}_{t}")
                for j in range(DRP):
                    nc.tensor.matmul(
                        pb[:],
                        lhsT=hT8_cur[t][:, 2 * j : 2 * j + 2, :],
                        rhs=wq[:, 2 * j : 2 * j + 2, :],
                        start=(j == 0),
                        stop=False,
                        perf_mode=DR,
                        skip_group_check=True,
                    )
                for kt in range(DRKT, KT):
                    nc.tensor.matmul(
                        pb[:],
                        lhsT=hT_cur[t][:, kt * 128 : (kt + 1) * 128],
                        rhs=wq[:, kt, :],
                        start=(DRP == 0 and kt == DRKT),
                        stop=(kt == KT - 1),
                        skip_group_check=True,
                    )
                # ScalarE PSUM reads must stay under one full 2048B bank
                o_t = outp.tile([128, cols], BF16, tag="ostg", name=f"o{q}_{t}")
                for c0 in range(0, cols, 256):
                    nc.scalar.activation(
                        o_t[:, c0 : c0 + 256], pb[:, c0 : c0 + 256],
                        AF.Copy, scale=float(head_scale),
                    )
                nc.sync.dma_start(
                    out[t * 128 : (t + 1) * 128, q * QV : q * QV + out_cols],
                    o_t[:, :out_cols],
                )

            # transition: T_head(t0) was emitted after the last layer's t1
            # matmuls; run q=0..NWARM-1 for t0 (covers t1's LN chain), then
            # T_head(t1), its warm groups, then the steady loop.
            for q in range(NWARM):
                head_group(q, 0, wq_pre[q], QV)
            emit_T(1, L)
            for q in range(NWARM):
                head_group(q, 1, wq_pre[q], QV)
            for q in range(NWARM, NQF):
                wq = wqp.tile([128, KT, QV], WDT, tag="wq", name=f"wq{q}")
                nc.sync.dma_start(wq[:], hw_[q])
                for t in range(TT):
                    head_group(q, t, wq, QV)
            # 256-wide vocab tail (32000 = 62*512 + 256)
            wqt = wqp.tile([128, KT, VTC], WDT, tag="wq", name="wqtail")
            nc.sync.dma_start(wqt[:], hwt_[:])
            for t in range(TT):
                head_group(NQF, t, wqt, VTC, out_cols=VT)

    return nc


def _ternary(wmat):
    """Exact {-1,0,1} ternary tensor + fp32 scale, matching the reference."""
    w = np.asarray(wmat, dtype=np.float32)
    s = np.mean(np.abs(w), dtype=np.float32)
    t = np.clip(np.rint(w / (s + np.float32(1e-8))), -1.0, 1.0).astype(np.float32)
    return t, float(s)


_NC_CACHE = {}
_LAST_RESULTS = None


def kernel(**inputs):
    global _LAST_RESULTS
    cfg = CFG_FULL
    L, H, NC, TT, V, QV, NQF, VT = (
        cfg["L"], cfg["H"], cfg["NC"], cfg["TT"], cfg["V"], cfg["QV"],
        cfg["NQF"], cfg["VT"],
    )
    KT = H // 128
    TPC = TT * 128  # tokens per core
    BF = ml_dtypes.bfloat16
    F8 = ml_dtypes.float8_e4m3fn
    fp8_w = not bool(int(os.environ.get("TRIKERNEL_BF16_W", "0")))
    use_dr = fp8_w and not bool(int(os.environ.get("TRIKERNEL_NO_DR", "0")))
    use_tail = bool(int(os.environ.get("TRIKERNEL_TAIL", "1")))
    WNP = F8 if fp8_w else BF

    ids = np.asarray(inputs["input_ids"]).astype(np.int64).reshape(-1)
    embed = np.asarray(inputs["embed"], dtype=np.float32)
    layer_w = np.asarray(inputs["layer_w"], dtype=np.float32)
    layer_b = np.asarray(inputs["layer_b"], dtype=np.float32)
    ln_g = np.asarray(inputs["ln_g"], dtype=np.float32)
    ln_b = np.asarray(inputs["ln_b"], dtype=np.float32)
    final_g = np.asarray(inputs["final_g"], dtype=np.float32)
    final_b = np.asarray(inputs["final_b"], dtype=np.float32)
    head_w = np.asarray(inputs["head_w"], dtype=np.float32)

    # trivial-affine specialization: the LN scale/shift and layer bias are
    # identity in this model instance; skip them on-chip when so.
    triv_ln = bool(
        np.all(ln_g == 1.0) and np.all(ln_b == 0.0) and np.all(layer_b == 0.0)
        and np.all(final_g == 1.0) and np.all(final_b == 0.0)
    )

    h0_full = embed[ids]  # [NTOK, H] fp32

    scales = []
    wT = np.empty([L, 128, KT, H], dtype=WNP)
    for l in range(L):
        t, s = _ternary(layer_w[l])
        scales.append(s)
        # [H(o), H(k)] -> transpose -> [KT,128,H] -> partition-major
        wT[l] = np.ascontiguousarray(
            t.T.reshape(KT, 128, H).transpose(1, 0, 2)
        ).astype(WNP)
    th, head_scale = _ternary(head_w)
    # head weights laid out so each [128, KT, QV] chunk is a single
    # contiguous 8KB-per-partition DMA: hw8[q, p, kt, v]; the 256-wide
    # vocab tail is its own tensor.
    thT = th.T  # [H, V]
    hw8 = np.ascontiguousarray(
        thT[:, : NQF * QV].reshape(KT, 128, NQF, QV).transpose(2, 1, 0, 3)
    ).astype(WNP)
    VTC = VT if use_tail else QV
    thT_tail = np.zeros((H, VTC), dtype=np.float32)
    thT_tail[:, :VT] = thT[:, NQF * QV :]
    hwt = np.ascontiguousarray(
        thT_tail.reshape(KT, 128, VTC).transpose(1, 0, 2)
    ).astype(WNP)

    key = (tuple(sorted(cfg.items())), tuple(scales), head_scale, triv_ln,
           fp8_w, use_dr, use_tail)
    if key not in _NC_CACHE:
        _NC_CACHE.clear()
        nc = build_nc(cfg, scales, head_scale, triv_ln, fp8_w, use_dr,
                      use_tail)
        # Bacc.finalize runs the TRN2 legalization passes (1-wait-per-
        # instruction event-semaphore split, matmul->ldweights wait motion,
        # register allocation). The PJRT exec path serializes nc as-is.
        nc.finalize()
        _NC_CACHE[key] = nc
    nc = _NC_CACHE[key]

    common = {
        "w": wT,
        "hw": hw8,
        "hwt": hwt,
        "identb": np.eye(128, dtype=BF),
        "eps": np.full((128, 1), EPS, np.float32),
    }
    if not triv_ln:
        common.update(
            lng=ln_g.astype(BF),
            lnb=ln_b.astype(BF),
            lbias=layer_b.astype(BF),
            fing=final_g.astype(BF),
            finb=final_b.astype(BF),
        )
    in_maps = []
    for c in range(NC):
        h0c = np.ascontiguousarray(
            h0_full[c * TPC : (c + 1) * TPC].reshape(TT, 128, H)
        )
        # host-side pre-transpose of the layer-0 lhsT (scaled, bf16)
        h0Tc = np.ascontiguousarray(
            (h0c.reshape(TT, 128, KT, 128).transpose(0, 3, 2, 1)
             * np.float32(scales[0])).reshape(TT, 128, H)
        ).astype(BF)
        in_maps.append(dict(common, h0=h0c.astype(BF), h0T=h0Tc))

    trace = bool(int(os.environ.get("TRIKERNEL_TRACE", "0")))
    res = run_bass_kernel_spmd(nc, in_maps, core_ids=list(range(NC)), trace=trace)
    _LAST_RESULTS = res

    full = np.concatenate(
        [np.asarray(res.results[c]["out"]).astype(np.float32) for c in range(NC)],
        axis=0,
    )  # [NTOK, V]
    return full.reshape(2, 1024, 32000)


# revision 45
# speedup vs baseline: 1.0091x; 1.0041x over previous
"""Trainium2 Bass kernel: 8-layer ternary (BitNet-1.58) dense transformer.

Model (per reference):
    h = embed[input_ids]                                  # (B=2, S=1024, H=2048)
    8x: y = h @ ternary(W_l)^T + b_l ; h = LN(y + h)*g+b  # H=2048
    h = LN(h)*final_g + final_b
    logits = h @ ternary(head_W)^T                        # (B, S, V=32000)

Sharding over 8 NeuronCores (fully local, no collectives):
  - Layers: data-parallel over the 2048 tokens (256 tokens/core). Each core
    streams the full ternary layer weights as exact {-1,0,+1} fp8(e4m3).
  - Head: ALSO data-parallel over tokens: each core computes its own 256
    tokens x the full 32000-entry vocab, streaming fp8 head weights
    chunk-by-chunk, overlapped with compute. No collectives at all.

Head matmul runs mixed precision: k-tiles 0..DRKT-1 via fp8 DoubleRow (2
k-tiles per instruction, activations rounded to e4m3), the rest as bf16
activations x fp8 weights at full precision. DRKT=8 costs ~1.91e-2 relative
error on the logits (vs the 2e-2 budget; host-simulated AND confirmed on HW)
and saves 2 of 13 matmul slots per vocab chunk vs DRKT=6.

The vocab is NOT padded on the compute side: 62 chunks of 512 plus one tail
chunk of 256 (32000 = 62*512 + 256), saving the 256 dead columns.

Schedule notes (from perfetto/NTFF analysis of the 640us baseline):
  - The PE stream is otherwise dense; the bubbles were (a) per-layer-tile
    ~0.8us waits of the activation transposes on the LayerNorm vector chain
    and (b) a 6.2us layers->head transition stall. Both are fixed by
    software-pipelining the transposes one stage deeper: the transpose
    feeding PE block N+1 is emitted right after block N's matmuls (and
    before the current LN's ScalarE squares, so its drain copies aren't
    queued behind them); ~14us of matmuls cover the ~6us LN chain latency.
    The first drain copy is kt0-only so the next block's first LDWEIGHTS
    unblocks ~250ns earlier.
  - Head transition: T_head(t0) is emitted inside the last layer;
    then q=0..NWARM-1 groups for t0 (covering t1's final LN chain),
    T_head(t1), its warm groups, then the steady loop. Head weight DMA for
    the warm chunks is issued before the last layer's matmuls.
  - The LN row-sum runs per 512-chunk right behind each z = psum + resid
    chunk (replacing one 2.3us full-row reduce). NB: DVE
    InstTensorTensorReduce faults on HW via this toolchain (both with PSUM
    and SBUF inputs) despite passing CoreSim -- do not use it.
  - bf16 logits + bf16 h0 keep the head phase under the per-core DMA
    ceiling (fp8 weight stream at ~200 GB/s/core + output writes); with
    fp32 logits the weight stream starves and the PE idles ~1.5us every
    few vocab chunks.
  - ~36 dummy matmuls on a memset scratch run during the initial DMA wait
    so the PE_HAM clock gate (1.2 -> 2.4 GHz after ~3.4us of activity) is
    released before the real stream starts.

When the LN affine params and biases are identity (they are for this model
instance; checked at runtime with a general fallback), the final LayerNorm
is also skipped: its input is already a LayerNorm output (per-token mean
exactly 0, variance 1-eps/var), so the final LN is an identity up to
O(eps)~2.5e-6.

HW notes (found the hard way): a ScalarE read of a full 2048B PSUM bank
hard-faults the exec unit -- all ScalarE PSUM reads here are <=1536B.
Activation transposes run as bf16 (2x faster through the PE than f32),
with a bf16 identity matrix as the moving operand.
"""

import os
import sys

import numpy as np

try:
    import concourse.bass as bass
except ImportError:  # grading container should have it on sys.path already
    sys.path.insert(0, "/opt/trn_rl_repo")
    import concourse.bass as bass

import ml_dtypes
import concourse.mybir as mybir
import concourse.tile as tile
from concourse import bacc
from concourse.bass_utils import run_bass_kernel_spmd
from contextlib import ExitStack

F32 = mybir.dt.float32
BF16 = mybir.dt.bfloat16
FP8E4 = mybir.dt.float8e4
AX = mybir.AxisListType
OP = mybir.AluOpType
AF = mybir.ActivationFunctionType
DR = mybir.MatmulPerfMode.DoubleRow
EPS = 1e-5

# Full-size problem config (B=2, S=1024 -> 2048 tokens, 256/core).
# Head: vocab = NQF full 512-chunks + one 256 tail; k-tiles 0..DRKT-1 run as
# fp8 DoubleRow.
CFG_FULL = dict(L=8, H=2048, NC=8, TT=2, V=32000, QV=512, NQF=62, VT=256,
                CH=512, DRKT=8)


def build_nc(cfg, scales, head_scale, triv_ln, fp8_w, use_dr, use_tail):
    L, H, NC, TT = cfg["L"], cfg["H"], cfg["NC"], cfg["TT"]
    V, QV, NQF, VT = cfg["V"], cfg["QV"], cfg["NQF"], cfg["VT"]
    CH, DRKT = cfg["CH"], cfg["DRKT"]
    KT = H // 128
    NCH = H // CH
    DRP = DRKT // 2
    if not use_dr:
        DRKT = DRP = 0
    assert H % CH == 0 and NQF * QV + VT == V
    WDT = FP8E4 if fp8_w else BF16

    nc = bacc.Bacc("TRN2", target_bir_lowering=False, debug=False, num_devices=NC)
    # h0 (layer-0 residual) ships as bf16: halves the startup DMA burst that
    # gates the first layer; costs ~3e-5 relative error on the logits.
    h0 = nc.declare_dram_parameter("h0", [TT, 128, H], BF16, isOutput=False)
    h0T = nc.declare_dram_parameter("h0T", [TT, 128, H], BF16, isOutput=False)
    # weights pre-arranged on host: [L, 128part, KT, H] -> contiguous
    # 8KB-per-partition quarter loads (fast DMA descriptor issue)
    w_ = nc.declare_dram_parameter("w", [L, 128, KT, H], WDT, isOutput=False)
    if not triv_ln:
        lng = nc.declare_dram_parameter("lng", [L, H], BF16, isOutput=False)
        lnb = nc.declare_dram_parameter("lnb", [L, H], BF16, isOutput=False)
        lbias = nc.declare_dram_parameter("lbias", [L, H], BF16, isOutput=False)
        fing = nc.declare_dram_parameter("fing", [H], BF16, isOutput=False)
        finb = nc.declare_dram_parameter("finb", [H], BF16, isOutput=False)
    hw_ = nc.declare_dram_parameter("hw", [NQF, 128, KT, QV], WDT, isOutput=False)
    VTC = VT if use_tail else QV  # tail compute width (QV = padded bisect mode)
    hwt_ = nc.declare_dram_parameter("hwt", [128, KT, VTC], WDT, isOutput=False)
    identb_d = nc.declare_dram_parameter("identb", [128, 128], BF16, isOutput=False)
    eps_d = nc.declare_dram_parameter("eps", [128, 1], F32, isOutput=False)
    # logits leave the core as bf16: halves the output DMA bytes (the head
    # phase is otherwise brushing the HBM ceiling: fp8 weight stream + fp32
    # logits + everything else) at +1e-4 relative error. Host upcasts.
    out = nc.declare_dram_parameter("out", [TT * 128, V], BF16, isOutput=True)

    with tile.TileContext(nc) as tc:
        with ExitStack() as ctx0:
            consts = ctx0.enter_context(tc.tile_pool(name="consts", bufs=1))
            state = ctx0.enter_context(tc.tile_pool(name="state", bufs=4))
            hTp = ctx0.enter_context(tc.tile_pool(name="hT", bufs=2))
            hT8p = ctx0.enter_context(tc.tile_pool(name="hT8", bufs=2))
            wqp = ctx0.enter_context(tc.tile_pool(name="wq", bufs=6))
            outp = ctx0.enter_context(tc.tile_pool(name="outstg", bufs=4))
            smp = ctx0.enter_context(tc.tile_pool(name="small", bufs=16))
            zpool = ctx0.enter_context(tc.tile_pool(name="z", bufs=2))
            wp = ctx0.enter_context(tc.tile_pool(name="w", bufs=12))
            sqp = ctx0.enter_context(tc.tile_pool(name="sq", bufs=2))
            hSp = ctx0.enter_context(tc.tile_pool(name="hS", bufs=2))
            gbp = None
            if not triv_ln:
                gbp = ctx0.enter_context(tc.tile_pool(name="gb", bufs=2))
            psT = ctx0.enter_context(tc.tile_pool(name="psT", bufs=2, space="PSUM"))
            # shared between layer output chunks and head vocab chunks
            psB = ctx0.enter_context(tc.tile_pool(name="psB", bufs=6, space="PSUM"))

            h_cur = []
            hT_cur = []
            for t in range(TT):
                hTt = hTp.tile([128, H], BF16, tag="hT", name=f"hT_p{t}")
                hT_cur.append(hTt)
                st = state.tile([128, H], BF16, name=f"hinit{t}", tag="state")
                h_cur.append(st)
            hT8_cur = [None] * TT
            hnS_pend = [None] * TT  # LN'd bf16 activations awaiting transpose

            def load_wq_sizes(l, sizes, defer=False):
                """Load layer l's weights in slices of sizes[i] k-tiles.
                Returns (tiles, kt_map) with kt_map[kt] = (slice_idx, local).
                defer=True skips the dma_start calls (caller issues them)."""
                tiles, kt_map, k0 = [], [], 0
                for hf, sz in enumerate(sizes):
                    wt = wp.tile([128, sz, H], WDT, tag="w", name=f"w{l}_{hf}")
                    if not defer:
                        nc.sync.dma_start(wt[:], w_[l, :, k0 : k0 + sz, :])
                    tiles.append(wt)
                    for j in range(sz):
                        kt_map.append((hf, j))
                    k0 += sz
                assert k0 == KT
                return tiles, kt_map

            def load_wq(l):
                return load_wq_sizes(l, [2] * 8)

            # DMA issue order tuned for time-to-first-matmul: layer-0 lhsT
            # tile and a single-k-tile first weight slice, then the rest;
            # h0/identb/eps are only needed several us later.
            sizes0 = [1, 1] + [2] * 7
            nc.sync.dma_start(hT_cur[0][:], h0T[0])
            w_pre = load_wq_sizes(0, sizes0, defer=True)
            w0_tiles = w_pre[0]
            k0s = [sum(sizes0[:i]) for i in range(len(sizes0))]
            for hf in range(1):
                nc.sync.dma_start(
                    w0_tiles[hf][:], w_[0, :, k0s[hf] : k0s[hf] + sizes0[hf], :]
                )
            nc.sync.dma_start(hT_cur[1][:], h0T[1])
            for hf in range(1, len(sizes0)):
                nc.sync.dma_start(
                    w0_tiles[hf][:], w_[0, :, k0s[hf] : k0s[hf] + sizes0[hf], :]
                )
            for t in range(TT):
                nc.sync.dma_start(h_cur[t][:], h0[t])
            identb = consts.tile([128, 128], BF16, name="identb")
            nc.sync.dma_start(identb[:], identb_d[:])
            eps_t = consts.tile([128, 1], F32, name="epst")
            nc.sync.dma_start(eps_t[:], eps_d[:])

            # PE warm-up: the HAM clock gate keeps the PE at 1.2 GHz until
            # ~3.4us of sustained activity. Run dummy transposes on a
            # memset scratch while the first DMAs are in flight so the
            # real matmul stream starts at 2.4 GHz.
            warm_src = consts.tile([128, 128], BF16, name="warmsrc")
            nc.gpsimd.memset(warm_src[:], 0.0)
            warm_ps = psT.tile([128, 128], F32, tag="psTb", name="warmps")
            for i in range(36):
                nc.tensor.matmul(
                    warm_ps[:], lhsT=warm_src[:], rhs=warm_src[:],
                    start=True, stop=True,
                )

            def transpose_bf(src_bf, name):
                """bf16 pre-scaled [128tok, H] -> hT bf16 [128feat-in-
                block, (kt,128tok)]; 1-bank bf16 psum halves, ScalarE
                psum reads kept at 1024B (full-bank reads fault)."""
                dst = hTp.tile([128, H], BF16, tag="hT", name=f"hT{name}")
                for hf in range(2):
                    pT = psT.tile(
                        [128, H // 2], BF16, tag="psTb", name=f"pT{name}_{hf}"
                    )
                    for k in range(KT // 2):
                        kt = hf * (KT // 2) + k
                        nc.tensor.transpose(
                            pT[:, k * 128 : (k + 1) * 128],
                            src_bf[:, kt * 128 : (kt + 1) * 128],
                            identb[:],
                        )
                    lo = hf * (H // 2)
                    if hf == 0:
                        # small first copy: kt0 alone unblocks the next MM
                        # block's first LDWEIGHTS ~250ns earlier
                        nc.scalar.copy(dst[:, lo : lo + 128], pT[:, :128])
                        nc.scalar.copy(dst[:, lo + 128 : lo + 512], pT[:, 128:512])
                    else:
                        nc.scalar.copy(dst[:, lo : lo + 512], pT[:, :512])
                    nc.scalar.copy(dst[:, lo + 512 : lo + 1024], pT[:, 512:])
                return dst

            def transpose_head(src_scaled_bf, src_unscaled_bf, name):
                """Final (head) lhsT: kt 0..DRKT-1 transposed from the
                UNSCALED bf16 activations -> e4m3 (DoubleRow operand);
                kt DRKT..15 from the head_scale-folded bf16 ones."""
                dst = hTp.tile([128, H], BF16, tag="hT", name=f"hT{name}")
                dst8 = None
                if DRKT:
                    dst8 = hT8p.tile(
                        [128, DRKT, 128], FP8E4, tag="hT8", name=f"hT8{name}"
                    )
                    d8v = dst8[:].rearrange("p a b -> p (a b)")
                    for c0 in range(0, DRKT, KT // 2):
                        grp = list(range(c0, min(c0 + KT // 2, DRKT)))
                        pX = psT.tile(
                            [128, H // 2], BF16, tag="psTb", name=f"pX{name}_{c0}"
                        )
                        for j, kt in enumerate(grp):
                            nc.tensor.transpose(
                                pX[:, j * 128 : (j + 1) * 128],
                                src_unscaled_bf[:, kt * 128 : (kt + 1) * 128],
                                identb[:],
                            )
                        n = len(grp) * 128
                        lo = c0 * 128
                        # ScalarE psum reads <=1536B: split if over 768 bf16;
                        # small first copy = first DR pair (kt0,kt1) so the
                        # head's first matmul unblocks early
                        splits = [256, 512, n] if c0 == 0 else [512, n]
                        prev = 0
                        for cut in splits:
                            cut = min(cut, n)
                            if cut > prev:
                                nc.scalar.copy(
                                    d8v[:, lo + prev : lo + cut],
                                    pX[:, prev:cut],
                                )
                            prev = cut
                rem = list(range(DRKT, KT))
                for c0 in range(0, len(rem), KT // 2):
                    grp = rem[c0 : c0 + KT // 2]
                    pT = psT.tile(
                        [128, H // 2], BF16, tag="psTb", name=f"pY{name}_{c0}"
                    )
                    for j, kt in enumerate(grp):
                        nc.tensor.transpose(
                            pT[:, j * 128 : (j + 1) * 128],
                            src_scaled_bf[:, kt * 128 : (kt + 1) * 128],
                            identb[:],
                        )
                    n = len(grp) * 128
                    lo = grp[0] * 128
                    half = min(512, n)
                    nc.scalar.copy(dst[:, lo : lo + half], pT[:, :half])
                    if half < n:
                        nc.scalar.copy(
                            dst[:, lo + half : lo + n], pT[:, half:n]
                        )
                return dst, dst8

            def ln_core(sums, name):
                """Shared LN statistics tail: returns (negmean, rstd)."""
                S = smp.tile([128, 1], F32, tag="s0", name=f"S{name}")
                SS = smp.tile([128, 1], F32, tag="s1", name=f"SS{name}")
                nc.vector.tensor_reduce(
                    S[:], sums[:, 0:NCH], axis=AX.X, op=OP.add
                )
                nc.vector.tensor_reduce(
                    SS[:], sums[:, NCH : 2 * NCH], axis=AX.X, op=OP.add
                )
                negmean = smp.tile([128, 1], F32, tag="s2", name=f"nm{name}")
                nc.vector.tensor_scalar_mul(negmean[:], S[:], -1.0 / H)
                msq = smp.tile([128, 1], F32, tag="s3", name=f"msq{name}")
                nc.vector.tensor_scalar_mul(msq[:], SS[:], 1.0 / H)
                var = smp.tile([128, 1], F32, tag="s4", name=f"var{name}")
                nc.vector.tensor_tensor(var[:], negmean[:], negmean[:], OP.mult)
                nc.vector.tensor_tensor(var[:], msq[:], var[:], OP.subtract)
                std = smp.tile([128, 1], F32, tag="s5", name=f"std{name}")
                nc.scalar.activation(std[:], var[:], AF.Sqrt, bias=eps_t[:])
                rstd = smp.tile([128, 1], F32, tag="s6", name=f"rstd{name}")
                nc.vector.reciprocal(rstd[:], std[:])
                return negmean, rstd

            fg = fb = None
            if not triv_ln:
                fg = gbp.tile([128, H], BF16, tag="g", name="gfin")
                nc.sync.dma_start(fg[:], fing[None, :].to_broadcast((128, H)))
                fb = gbp.tile([128, H], BF16, tag="b", name="bfin")
                nc.sync.dma_start(fb[:], finb[None, :].to_broadcast((128, H)))

            def emit_T(tn, ln):
                """Transpose tile tn's pending LN output into its lhsT for
                stage ln (a layer, or the head when ln == L). Emitted right
                after the MM block that precedes its consumer, and BEFORE
                the current LN's square ops, so its ScalarE drain copies
                aren't queued behind them."""
                if ln < L:
                    hT_cur[tn] = transpose_bf(hnS_pend[tn], f"{ln}_{tn}")
                else:
                    hT_cur[tn], hT8_cur[tn] = transpose_head(
                        hnS_pend[tn], hnS_pend[tn], f"fin{tn}"
                    )

            NWARM = 3  # head chunks prefetched + t0 groups before T_head(t1)
            wq_pre = []  # first head chunks, DMA'd under the last layer
            for l in range(L):
                w_q, kt_map = w_pre if l == 0 else load_wq(l)
                if l == L - 1:
                    for q in range(NWARM):
                        wq = wqp.tile(
                            [128, KT, QV], WDT, tag="wq", name=f"wqpre{q}"
                        )
                        nc.sync.dma_start(wq[:], hw_[q])
                        wq_pre.append(wq)
                g_t = b_t = bias_t = None
                if not triv_ln:
                    g_t = gbp.tile([128, H], BF16, tag="g", name=f"g{l}")
                    nc.sync.dma_start(
                        g_t[:], lng[l][None, :].to_broadcast((128, H))
                    )
                    b_t = gbp.tile([128, H], BF16, tag="b", name=f"b{l}")
                    nc.sync.dma_start(
                        b_t[:], lnb[l][None, :].to_broadcast((128, H))
                    )
                    bias_t = gbp.tile(
                        [128, H], BF16, tag="bias", name=f"bias{l}"
                    )
                    nc.sync.dma_start(
                        bias_t[:], lbias[l][None, :].to_broadcast((128, H))
                    )

                last = l == L - 1
                next_scale = scales[l + 1] if not last else head_scale
                for t in range(TT):
                    hTt = hT_cur[t]
                    ps = [
                        psB.tile([128, CH], F32, tag="psB", name=f"ps{l}_{t}_{i}")
                        for i in range(NCH)
                    ]
                    for half in range(2):
                        for kt in range(KT):
                            hf, loc = kt_map[kt]
                            wt = w_q[hf]
                            for i in (2 * half, 2 * half + 1):
                                nc.tensor.matmul(
                                    ps[i][:],
                                    lhsT=hTt[:, kt * 128 : (kt + 1) * 128],
                                    rhs=wt[:, loc, i * CH : (i + 1) * CH],
                                    start=(kt == 0),
                                    stop=(kt == KT - 1),
                                )
                    # transpose feeding the NEXT PE block, one stage deep:
                    # after MMs(l,t0) -> T for (l,t1); after MMs(l,t1) ->
                    # T for (l+1,t0) (l+1 == L -> the head's t0 operands).
                    if t == 0:
                        if l > 0:
                            emit_T(1, l)
                    elif l + 1 < L or triv_ln:
                        # non-triv: the head operands come from the true
                        # final LN emitted after the loop, not from here
                        emit_T(0, l + 1)
                    z = zpool.tile([128, H], F32, tag="z", name=f"z{l}_{t}")
                    sums = smp.tile(
                        [128, 2 * NCH], F32, tag="sums", name=f"sm{l}_{t}"
                    )
                    resid = h_cur[t]
                    if not triv_ln:
                        hb = zpool.tile(
                            [128, H], F32, tag="hb", name=f"hb{l}_{t}"
                        )
                        nc.vector.tensor_tensor(
                            hb[:], h_cur[t][:], bias_t[:], OP.add
                        )
                        resid = hb
                    # z = psum + resid per chunk, then per-chunk row-sums
                    # (a single fused tensor_tensor_reduce with a PSUM
                    # input faults on HW despite passing CoreSim).
                    for i in range(NCH):
                        nc.vector.tensor_add(
                            z[:, i * CH : (i + 1) * CH],
                            ps[i][:],
                            resid[:, i * CH : (i + 1) * CH],
                        )
                        nc.vector.tensor_reduce(
                            sums[:, i : i + 1],
                            z[:, i * CH : (i + 1) * CH],
                            axis=AX.X,
                            op=OP.add,
                        )
                    for i in range(NCH):
                        # sum-of-squares on the otherwise-idle GpSimd engine
                        # (SBUF-only), keeping ScalarE free for the transpose
                        # drain copies at the block boundary
                        sq = sqp.tile(
                            [128, CH], F32, tag="sq", name=f"sq{l}_{t}_{i}"
                        )
                        nc.gpsimd.tensor_tensor(
                            sq[:],
                            z[:, i * CH : (i + 1) * CH],
                            z[:, i * CH : (i + 1) * CH],
                            OP.mult,
                        )
                        nc.vector.tensor_reduce(
                            sums[:, NCH + i : NCH + i + 1],
                            sq[:],
                            axis=AX.X,
                            op=OP.add,
                        )
                    negmean, rstd = ln_core(sums, f"{l}_{t}")
                    rstd_s = smp.tile(
                        [128, 1], F32, tag="s7", name=f"rs{l}_{t}"
                    )
                    nc.vector.tensor_scalar_mul(
                        rstd_s[:], rstd[:], float(next_scale)
                    )

                    if triv_ln and last:
                        # hn is mean-0/var-1 by construction, so the
                        # final identity-affine LN is a no-op to O(eps):
                        # emit one UNSCALED bf16 activation feeding both
                        # head operand sets -- no f32 state, no stats.
                        hnS = hSp.tile(
                            [128, H], BF16, tag="hS", name=f"hS{l}_{t}"
                        )
                        nc.vector.tensor_scalar(
                            hnS[:], z[:], negmean[:], rstd[:],
                            OP.add, OP.mult,
                        )
                        hnS_pend[t] = hnS
                        continue

                    hn = state.tile([128, H], F32, tag="state", name=f"h{l}_{t}")
                    hnS = hSp.tile(
                        [128, H], BF16, tag="hS", name=f"hS{l}_{t}"
                    )
                    if triv_ln:
                        # hnS first: it feeds the transpose (latency-
                        # critical); the f32 state is only read next layer.
                        nc.vector.tensor_scalar(
                            hnS[:], z[:], negmean[:], rstd_s[:],
                            OP.add, OP.mult,
                        )
                        nc.vector.tensor_scalar(
                            hn[:], z[:], negmean[:], rstd[:], OP.add, OP.mult
                        )
                    else:
                        nc.vector.tensor_scalar(
                            hn[:], z[:], negmean[:], rstd[:], OP.add, OP.mult
                        )
                        nc.vector.tensor_tensor(hn[:], hn[:], g_t[:], OP.mult)
                        nc.vector.tensor_tensor(hn[:], hn[:], b_t[:], OP.add)
                        nc.vector.tensor_scalar_mul(
                            hnS[:], hn[:], float(next_scale)
                        )
                    h_cur[t] = hn
                    hnS_pend[t] = hnS

            if not triv_ln:
                # general path: true final LN over the f32 state
                for t in range(TT):
                    h8 = h_cur[t]
                    sums = smp.tile(
                        [128, 2 * NCH], F32, tag="sums", name=f"smf{t}"
                    )
                    nc.vector.tensor_reduce(
                        sums[:, 0:1], h8[:], axis=AX.X, op=OP.add
                    )
                    # only sums[:,0] used for S in this path
                    for i in range(1, NCH):
                        nc.vector.tensor_scalar_mul(
                            sums[:, i : i + 1], sums[:, 0:1], 0.0
                        )
                    for i in range(NCH):
                        sq = sqp.tile(
                            [128, CH], BF16, tag="sq", name=f"sqf{t}_{i}"
                        )
                        nc.scalar.activation(
                            sq[:],
                            h8[:, i * CH : (i + 1) * CH],
                            AF.Square,
                            accum_out=sums[:, NCH + i : NCH + i + 1],
                        )
                    negmean, rstd = ln_core(sums, f"fin{t}")
                    hfin = state.tile(
                        [128, H], F32, tag="state", name=f"hf{t}"
                    )
                    nc.vector.tensor_scalar(
                        hfin[:], h8[:], negmean[:], rstd[:], OP.add, OP.mult
                    )
                    nc.vector.tensor_tensor(hfin[:], hfin[:], fg[:], OP.mult)
                    nc.vector.tensor_tensor(hfin[:], hfin[:], fb[:], OP.add)
                    hnS = hSp.tile([128, H], BF16, tag="hS", name=f"hSf{t}")
                    nc.vector.tensor_scalar_mul(hnS[:], hfin[:], 1.0)
                    hnS_pend[t] = hnS
                if not triv_ln:
                    emit_T(0, L)

            # ---- head: own 256 tokens x full vocab, streamed fp8 weights ----
            def head_group(q, t, wq, cols, out_cols=None):
                out_cols = cols if out_cols is None else out_cols
                # single accumulation group: DR (e4m3) k-tiles then bf16
                # k-tiles into one bank; both lhsT operands are UNSCALED,
                # the ternary head scale is applied once at drain.
                pb = psB.tile([128, cols], F32, tag="psB", name=f"pb{q}_{t}")
                for j in range(DRP):
                    nc.tensor.matmul(
                        pb[:],
                        lhsT=hT8_cur[t][:, 2 * j : 2 * j + 2, :],
                        rhs=wq[:, 2 * j : 2 * j + 2, :],
                        start=(j == 0),
                        stop=False,
                        perf_mode=DR,
                        skip_group_check=True,
                    )
                for kt in range(DRKT, KT):
                    nc.tensor.matmul(
                        pb[:],
                        lhsT=hT_cur[t][:, kt * 128 : (kt + 1) * 128],
                        rhs=wq[:, kt, :],
                        start=(DRP == 0 and kt == DRKT),
                        stop=(kt == KT - 1),
                        skip_group_check=True,
                    )
                # ScalarE PSUM reads must stay under one full 2048B bank
                o_t = outp.tile([128, cols], BF16, tag="ostg", name=f"o{q}_{t}")
                for c0 in range(0, cols, 256):
                    nc.scalar.activation(
                        o_t[:, c0 : c0 + 256], pb[:, c0 : c0 + 256],
                        AF.Copy, scale=float(head_scale),
                    )
                nc.sync.dma_start(
                    out[t * 128 : (t + 1) * 128, q * QV : q * QV + out_cols],
                    o_t[:, :out_cols],
                )

            # transition: T_head(t0) was emitted after the last layer's t1
            # matmuls; run q=0..NWARM-1 for t0 (covers t1's LN chain), then
            # T_head(t1), its warm groups, then the steady loop.
            for q in range(NWARM):
                head_group(q, 0, wq_pre[q], QV)
            emit_T(1, L)
            for q in range(NWARM):
                head_group(q, 1, wq_pre[q], QV)
            for q in range(NWARM, NQF):
                wq = wqp.tile([128, KT, QV], WDT, tag="wq", name=f"wq{q}")
                nc.sync.dma_start(wq[:], hw_[q])
                for t in range(TT):
                    head_group(q, t, wq, QV)
            # 256-wide vocab tail (32000 = 62*512 + 256)
            wqt = wqp.tile([128, KT, VTC], WDT, tag="wq", name="wqtail")
            nc.sync.dma_start(wqt[:], hwt_[:])
            for t in range(TT):
                head_group(NQF, t, wqt, VTC, out_cols=VT)

    return nc


def _ternary(wmat):
    """Exact {-1,0,1} ternary tensor + fp32 scale, matching the reference."""
    w = np.asarray(wmat, dtype=np.float32)
    s = np.mean(np.abs(w), dtype=np.float32)
    t = np.clip(np.rint(w / (s + np.float32(1e-8))), -1.0, 1.0).astype(np.float32)
    return t, float(s)


_NC_CACHE = {}
_LAST_RESULTS = None


def kernel(**inputs):
    global _LAST_RESULTS
    cfg = CFG_FULL
    L, H, NC, TT, V, QV, NQF, VT = (
        cfg["L"], cfg["H"], cfg["NC"], cfg["TT"], cfg["V"], cfg["QV"],
        cfg["NQF"], cfg["VT"],
    )
    KT = H // 128
    TPC = TT * 128  # tokens per core
    BF = ml_dtypes.bfloat16
    F8 = ml_dtypes.float8_e4m3fn
    fp8_w = not bool(int(os.environ.get("TRIKERNEL_BF16_W", "0")))
    use_dr = fp8_w and not bool(int(os.environ.get("TRIKERNEL_NO_DR", "0")))
    use_tail = bool(int(os.environ.get("TRIKERNEL_TAIL", "1")))
    WNP = F8 if fp8_w else BF

    ids = np.asarray(inputs["input_ids"]).astype(np.int64).reshape(-1)
    embed = np.asarray(inputs["embed"], dtype=np.float32)
    layer_w = np.asarray(inputs["layer_w"], dtype=np.float32)
    layer_b = np.asarray(inputs["layer_b"], dtype=np.float32)
    ln_g = np.asarray(inputs["ln_g"], dtype=np.float32)
    ln_b = np.asarray(inputs["ln_b"], dtype=np.float32)
    final_g = np.asarray(inputs["final_g"], dtype=np.float32)
    final_b = np.asarray(inputs["final_b"], dtype=np.float32)
    head_w = np.asarray(inputs["head_w"], dtype=np.float32)

    # trivial-affine specialization: the LN scale/shift and layer bias are
    # identity in this model instance; skip them on-chip when so.
    triv_ln = bool(
        np.all(ln_g == 1.0) and np.all(ln_b == 0.0) and np.all(layer_b == 0.0)
        and np.all(final_g == 1.0) and np.all(final_b == 0.0)
    )

    h0_full = embed[ids]  # [NTOK, H] fp32

    scales = []
    wT = np.empty([L, 128, KT, H], dtype=WNP)
    for l in range(L):
        t, s = _ternary(layer_w[l])
        scales.append(s)
        # [H(o), H(k)] -> transpose -> [KT,128,H] -> partition-major
        wT[l] = np.ascontiguousarray(
            t.T.reshape(KT, 128, H).transpose(1, 0, 2)
        ).astype(WNP)
    th, head_scale = _ternary(head_w)
    # head weights laid out so each [128, KT, QV] chunk is a single
    # contiguous 8KB-per-partition DMA: hw8[q, p, kt, v]; the 256-wide
    # vocab tail is its own tensor.
    thT = th.T  # [H, V]
    hw8 = np.ascontiguousarray(
        thT[:, : NQF * QV].reshape(KT, 128, NQF, QV).transpose(2, 1, 0, 3)
    ).astype(WNP)
    VTC = VT if use_tail else QV
    thT_tail = np.zeros((H, VTC), dtype=np.float32)
    thT_tail[:, :VT] = thT[:, NQF * QV :]
    hwt = np.ascontiguousarray(
        thT_tail.reshape(KT, 128, VTC).transpose(1, 0, 2)
    ).astype(WNP)

    key = (tuple(sorted(cfg.items())), tuple(scales), head_scale, triv_ln,
           fp8_w, use_dr, use_tail)
    if key not in _NC_CACHE:
        _NC_CACHE.clear()
        nc = build_nc(cfg, scales, head_scale, triv_ln, fp8_w, use_dr,
                      use_tail)
        # Bacc.finalize runs the TRN2 legalization passes (1-wait-per-
        # instruction event-semaphore split, matmul->ldweights wait motion,
        # register allocation). The PJRT exec path serializes nc as-is.
        nc.finalize()
        _NC_CACHE[key] = nc
    nc = _NC_CACHE[key]

    common = {
        "w": wT,
        "hw": hw8,
        "hwt": hwt,
        "identb": np.eye(128, dtype=BF),
        "eps": np.full((128, 1), EPS, np.float32),
    }
    if not triv_ln:
        common.update(
            lng=ln_g.astype(BF),
            lnb=ln_b.astype(BF),
            lbias=layer_b.astype(BF),
            fing=final_g.astype(BF),
            finb=final_b.astype(BF),
        )
    in_maps = []
    for c in range(NC):
        h0c = np.ascontiguousarray(
            h0_full[c * TPC : (c + 1) * TPC].reshape(TT, 128, H)
        )
        # host-side pre-transpose of the layer-0 lhsT (scaled, bf16)
        h0Tc = np.ascontiguousarray(
            (h0c.reshape(TT, 128, KT, 128).transpose(0, 3, 2, 1)
             * np.float32(scales[0])).reshape(TT, 128, H)
        ).astype(BF)
        in_maps.append(dict(common, h0=h0c.astype(BF), h0T=h0Tc))

    trace = bool(int(os.environ.get("TRIKERNEL_TRACE", "0")))
    res = run_bass_kernel_spmd(nc, in_maps, core_ids=list(range(NC)), trace=trace)
    _LAST_RESULTS = res

    full = np.concatenate(
        [np.asarray(res.results[c]["out"]).astype(np.float32) for c in range(NC)],
        axis=0,
    )  # [NTOK, V]
    return full.reshape(2, 1024, 32000)


# revision 46
# speedup vs baseline: 1.0163x; 1.0072x over previous
"""Trainium2 Bass kernel: 8-layer ternary (BitNet-1.58) dense transformer.

Model (per reference):
    h = embed[input_ids]                                  # (B=2, S=1024, H=2048)
    8x: y = h @ ternary(W_l)^T + b_l ; h = LN(y + h)*g+b  # H=2048
    h = LN(h)*final_g + final_b
    logits = h @ ternary(head_W)^T                        # (B, S, V=32000)

Sharding over 8 NeuronCores (fully local, no collectives):
  - Layers: data-parallel over the 2048 tokens (256 tokens/core). Each core
    streams the full ternary layer weights as exact {-1,0,+1} fp8(e4m3).
  - Head: ALSO data-parallel over tokens: each core computes its own 256
    tokens x the full 32000-entry vocab, streaming fp8 head weights
    chunk-by-chunk, overlapped with compute. No collectives at all.

Head matmul runs mixed precision: k-tiles 0..DRKT-1 via fp8 DoubleRow (2
k-tiles per instruction, activations rounded to e4m3), the rest as bf16
activations x fp8 weights at full precision. DRKT=8 costs ~1.91e-2 relative
error on the logits (vs the 2e-2 budget; host-simulated AND confirmed on HW)
and saves 2 of 13 matmul slots per vocab chunk vs DRKT=6.

The vocab is NOT padded on the compute side: 62 chunks of 512 plus one tail
chunk of 256 (32000 = 62*512 + 256), saving the 256 dead columns.

Schedule notes (from perfetto/NTFF analysis of the 640us baseline):
  - The PE stream is otherwise dense; the bubbles were (a) per-layer-tile
    ~0.8us waits of the activation transposes on the LayerNorm vector chain
    and (b) a 6.2us layers->head transition stall. Both are fixed by
    software-pipelining the transposes one stage deeper: the transpose
    feeding PE block N+1 is emitted right after block N's matmuls (and
    before the current LN's ScalarE squares, so its drain copies aren't
    queued behind them); ~14us of matmuls cover the ~6us LN chain latency.
    The first drain copy is kt0-only so the next block's first LDWEIGHTS
    unblocks ~250ns earlier.
  - Head transition: T_head(t0) is emitted inside the last layer;
    then q=0..NWARM-1 groups for t0 (covering t1's final LN chain),
    T_head(t1), its warm groups, then the steady loop. Head weight DMA for
    the warm chunks is issued before the last layer's matmuls.
  - The LN row-sum runs per 512-chunk right behind each z = psum + resid
    chunk (replacing one 2.3us full-row reduce). NB: DVE
    InstTensorTensorReduce faults on HW via this toolchain (both with PSUM
    and SBUF inputs) despite passing CoreSim -- do not use it.
  - bf16 logits + bf16 h0 keep the head phase under the per-core DMA
    ceiling (fp8 weight stream at ~200 GB/s/core + output writes); with
    fp32 logits the weight stream starves and the PE idles ~1.5us every
    few vocab chunks.
  - ~36 dummy matmuls on a memset scratch run during the initial DMA wait
    so the PE_HAM clock gate (1.2 -> 2.4 GHz after ~3.4us of activity) is
    released before the real stream starts.

When the LN affine params and biases are identity (they are for this model
instance; checked at runtime with a general fallback), the final LayerNorm
is also skipped: its input is already a LayerNorm output (per-token mean
exactly 0, variance 1-eps/var), so the final LN is an identity up to
O(eps)~2.5e-6.

HW notes (found the hard way): a ScalarE read of a full 2048B PSUM bank
hard-faults the exec unit -- all ScalarE PSUM reads here are <=1536B.
Activation transposes run as bf16 (2x faster through the PE than f32),
with a bf16 identity matrix as the moving operand.
"""

import os
import sys

import numpy as np

try:
    import concourse.bass as bass
except ImportError:  # grading container should have it on sys.path already
    sys.path.insert(0, "/opt/trn_rl_repo")
    import concourse.bass as bass

import ml_dtypes
import concourse.mybir as mybir
import concourse.tile as tile
from concourse import bacc
from concourse.bass_utils import run_bass_kernel_spmd
from contextlib import ExitStack

F32 = mybir.dt.float32
BF16 = mybir.dt.bfloat16
FP8E4 = mybir.dt.float8e4
AX = mybir.AxisListType
OP = mybir.AluOpType
AF = mybir.ActivationFunctionType
DR = mybir.MatmulPerfMode.DoubleRow
EPS = 1e-5

# Full-size problem config (B=2, S=1024 -> 2048 tokens, 256/core).
# Head: vocab = NQF full 512-chunks + one 256 tail; k-tiles 0..DRKT-1 run as
# fp8 DoubleRow.
CFG_FULL = dict(L=8, H=2048, NC=8, TT=2, V=32000, QV=512, NQF=62, VT=256,
                CH=512, DRKT=8)


def build_nc(cfg, scales, head_scale, triv_ln, fp8_w, use_dr, use_tail):
    L, H, NC, TT = cfg["L"], cfg["H"], cfg["NC"], cfg["TT"]
    V, QV, NQF, VT = cfg["V"], cfg["QV"], cfg["NQF"], cfg["VT"]
    CH, DRKT = cfg["CH"], cfg["DRKT"]
    KT = H // 128
    NCH = H // CH
    DRP = DRKT // 2
    if not use_dr:
        DRKT = DRP = 0
    assert H % CH == 0 and NQF * QV + VT == V
    WDT = FP8E4 if fp8_w else BF16

    nc = bacc.Bacc("TRN2", target_bir_lowering=False, debug=False, num_devices=NC)
    # h0 (layer-0 residual) ships as bf16: halves the startup DMA burst that
    # gates the first layer; costs ~3e-5 relative error on the logits.
    h0 = nc.declare_dram_parameter("h0", [TT, 128, H], BF16, isOutput=False)
    h0T = nc.declare_dram_parameter("h0T", [TT, 128, H], BF16, isOutput=False)
    # weights pre-arranged on host: [L, 128part, KT, H] -> contiguous
    # 8KB-per-partition quarter loads (fast DMA descriptor issue)
    w_ = nc.declare_dram_parameter("w", [L, 128, KT, H], WDT, isOutput=False)
    if not triv_ln:
        lng = nc.declare_dram_parameter("lng", [L, H], BF16, isOutput=False)
        lnb = nc.declare_dram_parameter("lnb", [L, H], BF16, isOutput=False)
        lbias = nc.declare_dram_parameter("lbias", [L, H], BF16, isOutput=False)
        fing = nc.declare_dram_parameter("fing", [H], BF16, isOutput=False)
        finb = nc.declare_dram_parameter("finb", [H], BF16, isOutput=False)
    hw_ = nc.declare_dram_parameter("hw", [NQF, 128, KT, QV], WDT, isOutput=False)
    VTC = VT if use_tail else QV  # tail compute width (QV = padded bisect mode)
    hwt_ = nc.declare_dram_parameter("hwt", [128, KT, VTC], WDT, isOutput=False)
    identb_d = nc.declare_dram_parameter("identb", [128, 128], BF16, isOutput=False)
    eps_d = nc.declare_dram_parameter("eps", [128, 1], F32, isOutput=False)
    # logits leave the core as bf16: halves the output DMA bytes (the head
    # phase is otherwise brushing the HBM ceiling: fp8 weight stream + fp32
    # logits + everything else) at +1e-4 relative error. Host upcasts.
    out = nc.declare_dram_parameter("out", [TT * 128, V], BF16, isOutput=True)

    with tile.TileContext(nc) as tc:
        with ExitStack() as ctx0:
            consts = ctx0.enter_context(tc.tile_pool(name="consts", bufs=1))
            state = ctx0.enter_context(tc.tile_pool(name="state", bufs=4))
            hTp = ctx0.enter_context(tc.tile_pool(name="hT", bufs=2))
            hT8p = ctx0.enter_context(tc.tile_pool(name="hT8", bufs=2))
            wqp = ctx0.enter_context(tc.tile_pool(name="wq", bufs=6))
            outp = ctx0.enter_context(tc.tile_pool(name="outstg", bufs=4))
            smp = ctx0.enter_context(tc.tile_pool(name="small", bufs=16))
            zpool = ctx0.enter_context(tc.tile_pool(name="z", bufs=2))
            wp = ctx0.enter_context(tc.tile_pool(name="w", bufs=12))
            sqp = ctx0.enter_context(tc.tile_pool(name="sq", bufs=2))
            hSp = ctx0.enter_context(tc.tile_pool(name="hS", bufs=2))
            gbp = None
            if not triv_ln:
                gbp = ctx0.enter_context(tc.tile_pool(name="gb", bufs=2))
            psT = ctx0.enter_context(tc.tile_pool(name="psT", bufs=2, space="PSUM"))
            # shared between layer output chunks and head vocab chunks
            psB = ctx0.enter_context(tc.tile_pool(name="psB", bufs=6, space="PSUM"))

            h_cur = []
            hT_cur = []
            for t in range(TT):
                hTt = hTp.tile([128, H], BF16, tag="hT", name=f"hT_p{t}")
                hT_cur.append(hTt)
                st = state.tile([128, H], BF16, name=f"hinit{t}", tag="state")
                h_cur.append(st)
            hT8_cur = [None] * TT
            hnS_pend = [None] * TT  # LN'd bf16 activations awaiting transpose

            def load_wq_sizes(l, sizes, defer=False):
                """Load layer l's weights in slices of sizes[i] k-tiles.
                Returns (tiles, kt_map) with kt_map[kt] = (slice_idx, local).
                defer=True skips the dma_start calls (caller issues them)."""
                tiles, kt_map, k0 = [], [], 0
                for hf, sz in enumerate(sizes):
                    wt = wp.tile([128, sz, H], WDT, tag="w", name=f"w{l}_{hf}")
                    if not defer:
                        nc.sync.dma_start(wt[:], w_[l, :, k0 : k0 + sz, :])
                    tiles.append(wt)
                    for j in range(sz):
                        kt_map.append((hf, j))
                    k0 += sz
                assert k0 == KT
                return tiles, kt_map

            def load_wq(l):
                return load_wq_sizes(l, [2] * 8)

            # DMA issue order tuned for time-to-first-matmul: layer-0 lhsT
            # tile and a single-k-tile first weight slice, then the rest;
            # h0/identb/eps are only needed several us later.
            sizes0 = [1, 1] + [2] * 7
            nc.sync.dma_start(hT_cur[0][:], h0T[0])
            w_pre = load_wq_sizes(0, sizes0, defer=True)
            w0_tiles = w_pre[0]
            k0s = [sum(sizes0[:i]) for i in range(len(sizes0))]
            for hf in range(1):
                nc.sync.dma_start(
                    w0_tiles[hf][:], w_[0, :, k0s[hf] : k0s[hf] + sizes0[hf], :]
                )
            nc.sync.dma_start(hT_cur[1][:], h0T[1])
            for hf in range(1, len(sizes0)):
                nc.sync.dma_start(
                    w0_tiles[hf][:], w_[0, :, k0s[hf] : k0s[hf] + sizes0[hf], :]
                )
            for t in range(TT):
                nc.sync.dma_start(h_cur[t][:], h0[t])
            identb = consts.tile([128, 128], BF16, name="identb")
            nc.sync.dma_start(identb[:], identb_d[:])
            eps_t = consts.tile([128, 1], F32, name="epst")
            nc.sync.dma_start(eps_t[:], eps_d[:])

            # PE warm-up: the HAM clock gate keeps the PE at 1.2 GHz until
            # ~3.4us of sustained activity. Run dummy transposes on a
            # memset scratch while the first DMAs are in flight so the
            # real matmul stream starts at 2.4 GHz.
            warm_src = consts.tile([128, 128], BF16, name="warmsrc")
            nc.gpsimd.memset(warm_src[:], 0.0)
            warm_ps = psT.tile([128, 128], F32, tag="psTb", name="warmps")
            for i in range(36):
                nc.tensor.matmul(
                    warm_ps[:], lhsT=warm_src[:], rhs=warm_src[:],
                    start=True, stop=True,
                )

            def transpose_bf(src_bf, name):
                """bf16 pre-scaled [128tok, H] -> hT bf16 [128feat-in-
                block, (kt,128tok)]; 1-bank bf16 psum halves, ScalarE
                psum reads kept at 1024B (full-bank reads fault)."""
                dst = hTp.tile([128, H], BF16, tag="hT", name=f"hT{name}")
                for hf in range(2):
                    pT = psT.tile(
                        [128, H // 2], BF16, tag="psTb", name=f"pT{name}_{hf}"
                    )
                    for k in range(KT // 2):
                        kt = hf * (KT // 2) + k
                        nc.tensor.transpose(
                            pT[:, k * 128 : (k + 1) * 128],
                            src_bf[:, kt * 128 : (kt + 1) * 128],
                            identb[:],
                        )
                    lo = hf * (H // 2)
                    if hf == 0:
                        # small first copy: kt0 alone unblocks the next MM
                        # block's first LDWEIGHTS ~250ns earlier
                        nc.scalar.copy(dst[:, lo : lo + 128], pT[:, :128])
                        nc.scalar.copy(dst[:, lo + 128 : lo + 512], pT[:, 128:512])
                    else:
                        nc.scalar.copy(dst[:, lo : lo + 512], pT[:, :512])
                    nc.scalar.copy(dst[:, lo + 512 : lo + 1024], pT[:, 512:])
                return dst

            def transpose_head(src_scaled_bf, src_unscaled_bf, name):
                """Final (head) lhsT: kt 0..DRKT-1 transposed from the
                UNSCALED bf16 activations -> e4m3 (DoubleRow operand);
                kt DRKT..15 from the head_scale-folded bf16 ones."""
                dst = hTp.tile([128, H], BF16, tag="hT", name=f"hT{name}")
                dst8 = None
                if DRKT:
                    dst8 = hT8p.tile(
                        [128, DRKT, 128], FP8E4, tag="hT8", name=f"hT8{name}"
                    )
                    d8v = dst8[:].rearrange("p a b -> p (a b)")
                    for c0 in range(0, DRKT, KT // 2):
                        grp = list(range(c0, min(c0 + KT // 2, DRKT)))
                        pX = psT.tile(
                            [128, H // 2], BF16, tag="psTb", name=f"pX{name}_{c0}"
                        )
                        for j, kt in enumerate(grp):
                            nc.tensor.transpose(
                                pX[:, j * 128 : (j + 1) * 128],
                                src_unscaled_bf[:, kt * 128 : (kt + 1) * 128],
                                identb[:],
                            )
                        n = len(grp) * 128
                        lo = c0 * 128
                        # ScalarE psum reads <=1536B: split if over 768 bf16;
                        # small first copy = first DR pair (kt0,kt1) so the
                        # head's first matmul unblocks early
                        splits = [256, 512, n] if c0 == 0 else [512, n]
                        prev = 0
                        for cut in splits:
                            cut = min(cut, n)
                            if cut > prev:
                                nc.scalar.copy(
                                    d8v[:, lo + prev : lo + cut],
                                    pX[:, prev:cut],
                                )
                            prev = cut
                rem = list(range(DRKT, KT))
                for c0 in range(0, len(rem), KT // 2):
                    grp = rem[c0 : c0 + KT // 2]
                    pT = psT.tile(
                        [128, H // 2], BF16, tag="psTb", name=f"pY{name}_{c0}"
                    )
                    for j, kt in enumerate(grp):
                        nc.tensor.transpose(
                            pT[:, j * 128 : (j + 1) * 128],
                            src_scaled_bf[:, kt * 128 : (kt + 1) * 128],
                            identb[:],
                        )
                    n = len(grp) * 128
                    lo = grp[0] * 128
                    half = min(512, n)
                    nc.scalar.copy(dst[:, lo : lo + half], pT[:, :half])
                    if half < n:
                        nc.scalar.copy(
                            dst[:, lo + half : lo + n], pT[:, half:n]
                        )
                return dst, dst8

            def ln_core(sums, name):
                """Shared LN statistics tail: returns (negmean, rstd)."""
                S = smp.tile([128, 1], F32, tag="s0", name=f"S{name}")
                SS = smp.tile([128, 1], F32, tag="s1", name=f"SS{name}")
                nc.vector.tensor_reduce(
                    S[:], sums[:, 0:NCH], axis=AX.X, op=OP.add
                )
                nc.vector.tensor_reduce(
                    SS[:], sums[:, NCH : 2 * NCH], axis=AX.X, op=OP.add
                )
                negmean = smp.tile([128, 1], F32, tag="s2", name=f"nm{name}")
                nc.vector.tensor_scalar_mul(negmean[:], S[:], -1.0 / H)
                msq = smp.tile([128, 1], F32, tag="s3", name=f"msq{name}")
                nc.vector.tensor_scalar_mul(msq[:], SS[:], 1.0 / H)
                var = smp.tile([128, 1], F32, tag="s4", name=f"var{name}")
                nc.vector.tensor_tensor(var[:], negmean[:], negmean[:], OP.mult)
                nc.vector.tensor_tensor(var[:], msq[:], var[:], OP.subtract)
                std = smp.tile([128, 1], F32, tag="s5", name=f"std{name}")
                nc.scalar.activation(std[:], var[:], AF.Sqrt, bias=eps_t[:])
                rstd = smp.tile([128, 1], F32, tag="s6", name=f"rstd{name}")
                nc.vector.reciprocal(rstd[:], std[:])
                return negmean, rstd

            fg = fb = None
            if not triv_ln:
                fg = gbp.tile([128, H], BF16, tag="g", name="gfin")
                nc.sync.dma_start(fg[:], fing[None, :].to_broadcast((128, H)))
                fb = gbp.tile([128, H], BF16, tag="b", name="bfin")
                nc.sync.dma_start(fb[:], finb[None, :].to_broadcast((128, H)))

            def emit_T(tn, ln):
                """Transpose tile tn's pending LN output into its lhsT for
                stage ln (a layer, or the head when ln == L). Emitted right
                after the MM block that precedes its consumer, and BEFORE
                the current LN's square ops, so its ScalarE drain copies
                aren't queued behind them."""
                if ln < L:
                    hT_cur[tn] = transpose_bf(hnS_pend[tn], f"{ln}_{tn}")
                else:
                    hT_cur[tn], hT8_cur[tn] = transpose_head(
                        hnS_pend[tn], hnS_pend[tn], f"fin{tn}"
                    )

            NWARM = 3  # head chunks prefetched + t0 groups before T_head(t1)
            wq_pre = []  # first head chunks, DMA'd under the last layer
            for l in range(L):
                w_q, kt_map = w_pre if l == 0 else load_wq(l)
                if l == L - 1:
                    for q in range(NWARM):
                        wq = wqp.tile(
                            [128, KT, QV], WDT, tag="wq", name=f"wqpre{q}"
                        )
                        nc.sync.dma_start(wq[:], hw_[q])
                        wq_pre.append(wq)
                g_t = b_t = bias_t = None
                if not triv_ln:
                    g_t = gbp.tile([128, H], BF16, tag="g", name=f"g{l}")
                    nc.sync.dma_start(
                        g_t[:], lng[l][None, :].to_broadcast((128, H))
                    )
                    b_t = gbp.tile([128, H], BF16, tag="b", name=f"b{l}")
                    nc.sync.dma_start(
                        b_t[:], lnb[l][None, :].to_broadcast((128, H))
                    )
                    bias_t = gbp.tile(
                        [128, H], BF16, tag="bias", name=f"bias{l}"
                    )
                    nc.sync.dma_start(
                        bias_t[:], lbias[l][None, :].to_broadcast((128, H))
                    )

                last = l == L - 1
                next_scale = scales[l + 1] if not last else head_scale
                for t in range(TT):
                    hTt = hT_cur[t]
                    ps = [
                        psB.tile([128, CH], F32, tag="psB", name=f"ps{l}_{t}_{i}")
                        for i in range(NCH)
                    ]
                    for half in range(2):
                        for kt in range(KT):
                            hf, loc = kt_map[kt]
                            wt = w_q[hf]
                            for i in (2 * half, 2 * half + 1):
                                nc.tensor.matmul(
                                    ps[i][:],
                                    lhsT=hTt[:, kt * 128 : (kt + 1) * 128],
                                    rhs=wt[:, loc, i * CH : (i + 1) * CH],
                                    start=(kt == 0),
                                    stop=(kt == KT - 1),
                                )
                    # transpose feeding the NEXT PE block, one stage deep:
                    # after MMs(l,t0) -> T for (l,t1); after MMs(l,t1) ->
                    # T for (l+1,t0) (l+1 == L -> the head's t0 operands).
                    if t == 0:
                        if l > 0:
                            emit_T(1, l)
                    elif l + 1 < L or triv_ln:
                        # non-triv: the head operands come from the true
                        # final LN emitted after the loop, not from here
                        emit_T(0, l + 1)
                    z = zpool.tile([128, H], F32, tag="z", name=f"z{l}_{t}")
                    sums = smp.tile(
                        [128, 2 * NCH], F32, tag="sums", name=f"sm{l}_{t}"
                    )
                    resid = h_cur[t]
                    if not triv_ln:
                        hb = zpool.tile(
                            [128, H], F32, tag="hb", name=f"hb{l}_{t}"
                        )
                        nc.vector.tensor_tensor(
                            hb[:], h_cur[t][:], bias_t[:], OP.add
                        )
                        resid = hb
                    # z = psum + resid per chunk, then per-chunk row-sums
                    # (a single fused tensor_tensor_reduce with a PSUM
                    # input faults on HW despite passing CoreSim).
                    for i in range(NCH):
                        nc.vector.tensor_add(
                            z[:, i * CH : (i + 1) * CH],
                            ps[i][:],
                            resid[:, i * CH : (i + 1) * CH],
                        )
                        nc.vector.tensor_reduce(
                            sums[:, i : i + 1],
                            z[:, i * CH : (i + 1) * CH],
                            axis=AX.X,
                            op=OP.add,
                        )
                    for i in range(NCH):
                        # sum-of-squares: first chunks on ScalarE (ready
                        # early, while the MM block still runs), last chunks
                        # on the otherwise-idle GpSimd so ScalarE is free
                        # for the transpose drain copies at the boundary
                        sq = sqp.tile(
                            [128, CH], F32, tag="sq", name=f"sq{l}_{t}_{i}"
                        )
                        if i < NCH // 2:
                            nc.scalar.activation(
                                sq[:],
                                z[:, i * CH : (i + 1) * CH],
                                AF.Square,
                                accum_out=sums[:, NCH + i : NCH + i + 1],
                            )
                        else:
                            nc.gpsimd.tensor_tensor(
                                sq[:],
                                z[:, i * CH : (i + 1) * CH],
                                z[:, i * CH : (i + 1) * CH],
                                OP.mult,
                            )
                            nc.vector.tensor_reduce(
                                sums[:, NCH + i : NCH + i + 1],
                                sq[:],
                                axis=AX.X,
                                op=OP.add,
                            )
                    negmean, rstd = ln_core(sums, f"{l}_{t}")
                    rstd_s = smp.tile(
                        [128, 1], F32, tag="s7", name=f"rs{l}_{t}"
                    )
                    nc.vector.tensor_scalar_mul(
                        rstd_s[:], rstd[:], float(next_scale)
                    )

                    if triv_ln and last:
                        # hn is mean-0/var-1 by construction, so the
                        # final identity-affine LN is a no-op to O(eps):
                        # emit one UNSCALED bf16 activation feeding both
                        # head operand sets -- no f32 state, no stats.
                        hnS = hSp.tile(
                            [128, H], BF16, tag="hS", name=f"hS{l}_{t}"
                        )
                        nc.vector.tensor_scalar(
                            hnS[:], z[:], negmean[:], rstd[:],
                            OP.add, OP.mult,
                        )
                        hnS_pend[t] = hnS
                        continue

                    hn = state.tile([128, H], F32, tag="state", name=f"h{l}_{t}")
                    hnS = hSp.tile(
                        [128, H], BF16, tag="hS", name=f"hS{l}_{t}"
                    )
                    if triv_ln:
                        # hnS first: it feeds the transpose (latency-
                        # critical); the f32 state is only read next layer.
                        nc.vector.tensor_scalar(
                            hnS[:], z[:], negmean[:], rstd_s[:],
                            OP.add, OP.mult,
                        )
                        nc.vector.tensor_scalar(
                            hn[:], z[:], negmean[:], rstd[:], OP.add, OP.mult
                        )
                    else:
                        nc.vector.tensor_scalar(
                            hn[:], z[:], negmean[:], rstd[:], OP.add, OP.mult
                        )
                        nc.vector.tensor_tensor(hn[:], hn[:], g_t[:], OP.mult)
                        nc.vector.tensor_tensor(hn[:], hn[:], b_t[:], OP.add)
                        nc.vector.tensor_scalar_mul(
                            hnS[:], hn[:], float(next_scale)
                        )
                    h_cur[t] = hn
                    hnS_pend[t] = hnS

            if not triv_ln:
                # general path: true final LN over the f32 state
                for t in range(TT):
                    h8 = h_cur[t]
                    sums = smp.tile(
                        [128, 2 * NCH], F32, tag="sums", name=f"smf{t}"
                    )
                    nc.vector.tensor_reduce(
                        sums[:, 0:1], h8[:], axis=AX.X, op=OP.add
                    )
                    # only sums[:,0] used for S in this path
                    for i in range(1, NCH):
                        nc.vector.tensor_scalar_mul(
                            sums[:, i : i + 1], sums[:, 0:1], 0.0
                        )
                    for i in range(NCH):
                        sq = sqp.tile(
                            [128, CH], BF16, tag="sq", name=f"sqf{t}_{i}"
                        )
                        nc.scalar.activation(
                            sq[:],
                            h8[:, i * CH : (i + 1) * CH],
                            AF.Square,
                            accum_out=sums[:, NCH + i : NCH + i + 1],
                        )
                    negmean, rstd = ln_core(sums, f"fin{t}")
                    hfin = state.tile(
                        [128, H], F32, tag="state", name=f"hf{t}"
                    )
                    nc.vector.tensor_scalar(
                        hfin[:], h8[:], negmean[:], rstd[:], OP.add, OP.mult
                    )
                    nc.vector.tensor_tensor(hfin[:], hfin[:], fg[:], OP.mult)
                    nc.vector.tensor_tensor(hfin[:], hfin[:], fb[:], OP.add)
                    hnS = hSp.tile([128, H], BF16, tag="hS", name=f"hSf{t}")
                    nc.vector.tensor_scalar_mul(hnS[:], hfin[:], 1.0)
                    hnS_pend[t] = hnS
                if not triv_ln:
                    emit_T(0, L)

            # ---- head: own 256 tokens x full vocab, streamed fp8 weights ----
            def head_group(q, t, wq, cols, out_cols=None):
                out_cols = cols if out_cols is None else out_cols
                # single accumulation group: DR (e4m3) k-tiles then bf16
                # k-tiles into one bank; both lhsT operands are UNSCALED,
                # the ternary head scale is applied once at drain.
                pb = psB.tile([128, cols], F32, tag="psB", name=f"pb{q}_{t}")
                for j in range(DRP):
                    nc.tensor.matmul(
                        pb[:],
                        lhsT=hT8_cur[t][:, 2 * j : 2 * j + 2, :],
                        rhs=wq[:, 2 * j : 2 * j + 2, :],
                        start=(j == 0),
                        stop=False,
                        perf_mode=DR,
                        skip_group_check=True,
                    )
                for kt in range(DRKT, KT):
                    nc.tensor.matmul(
                        pb[:],
                        lhsT=hT_cur[t][:, kt * 128 : (kt + 1) * 128],
                        rhs=wq[:, kt, :],
                        start=(DRP == 0 and kt == DRKT),
                        stop=(kt == KT - 1),
                        skip_group_check=True,
                    )
                # ScalarE PSUM reads must stay under one full 2048B bank
                o_t = outp.tile([128, cols], BF16, tag="ostg", name=f"o{q}_{t}")
                for c0 in range(0, cols, 256):
                    nc.scalar.activation(
                        o_t[:, c0 : c0 + 256], pb[:, c0 : c0 + 256],
                        AF.Copy, scale=float(head_scale),
                    )
                nc.sync.dma_start(
                    out[t * 128 : (t + 1) * 128, q * QV : q * QV + out_cols],
                    o_t[:, :out_cols],
                )

            # transition: T_head(t0) was emitted after the last layer's t1
            # matmuls; run q=0..NWARM-1 for t0 (covers t1's LN chain), then
            # T_head(t1), its warm groups, then the steady loop.
            for q in range(NWARM):
                head_group(q, 0, wq_pre[q], QV)
            emit_T(1, L)
            for q in range(NWARM):
                head_group(q, 1, wq_pre[q], QV)
            for q in range(NWARM, NQF):
                wq = wqp.tile([128, KT, QV], WDT, tag="wq", name=f"wq{q}")
                nc.sync.dma_start(wq[:], hw_[q])
                for t in range(TT):
                    head_group(q, t, wq, QV)
            # 256-wide vocab tail (32000 = 62*512 + 256)
            wqt = wqp.tile([128, KT, VTC], WDT, tag="wq", name="wqtail")
            nc.sync.dma_start(wqt[:], hwt_[:])
            for t in range(TT):
                head_group(NQF, t, wqt, VTC, out_cols=VT)

    return nc


def _ternary(wmat):
    """Exact {-1,0,1} ternary tensor + fp32 scale, matching the reference."""
    w = np.asarray(wmat, dtype=np.float32)
    s = np.mean(np.abs(w), dtype=np.float32)
    t = np.clip(np.rint(w / (s + np.float32(1e-8))), -1.0, 1.0).astype(np.float32)
    return t, float(s)


_NC_CACHE = {}
_LAST_RESULTS = None


def kernel(**inputs):
    global _LAST_RESULTS
    cfg = CFG_FULL
    L, H, NC, TT, V, QV, NQF, VT = (
        cfg["L"], cfg["H"], cfg["NC"], cfg["TT"], cfg["V"], cfg["QV"],
        cfg["NQF"], cfg["VT"],
    )
    KT = H // 128
    TPC = TT * 128  # tokens per core
    BF = ml_dtypes.bfloat16
    F8 = ml_dtypes.float8_e4m3fn
    fp8_w = not bool(int(os.environ.get("TRIKERNEL_BF16_W", "0")))
    use_dr = fp8_w and not bool(int(os.environ.get("TRIKERNEL_NO_DR", "0")))
    use_tail = bool(int(os.environ.get("TRIKERNEL_TAIL", "1")))
    WNP = F8 if fp8_w else BF

    ids = np.asarray(inputs["input_ids"]).astype(np.int64).reshape(-1)
    embed = np.asarray(inputs["embed"], dtype=np.float32)
    layer_w = np.asarray(inputs["layer_w"], dtype=np.float32)
    layer_b = np.asarray(inputs["layer_b"], dtype=np.float32)
    ln_g = np.asarray(inputs["ln_g"], dtype=np.float32)
    ln_b = np.asarray(inputs["ln_b"], dtype=np.float32)
    final_g = np.asarray(inputs["final_g"], dtype=np.float32)
    final_b = np.asarray(inputs["final_b"], dtype=np.float32)
    head_w = np.asarray(inputs["head_w"], dtype=np.float32)

    # trivial-affine specialization: the LN scale/shift and layer bias are
    # identity in this model instance; skip them on-chip when so.
    triv_ln = bool(
        np.all(ln_g == 1.0) and np.all(ln_b == 0.0) and np.all(layer_b == 0.0)
        and np.all(final_g == 1.0) and np.all(final_b == 0.0)
    )

    h0_full = embed[ids]  # [NTOK, H] fp32

    scales = []
    wT = np.empty([L, 128, KT, H], dtype=WNP)
    for l in range(L):
        t, s = _ternary(layer_w[l])
        scales.append(s)
        # [H(o), H(k)] -> transpose -> [KT,128,H] -> partition-major
        wT[l] = np.ascontiguousarray(
            t.T.reshape(KT, 128, H).transpose(1, 0, 2)
        ).astype(WNP)
    th, head_scale = _ternary(head_w)
    # head weights laid out so each [128, KT, QV] chunk is a single
    # contiguous 8KB-per-partition DMA: hw8[q, p, kt, v]; the 256-wide
    # vocab tail is its own tensor.
    thT = th.T  # [H, V]
    hw8 = np.ascontiguousarray(
        thT[:, : NQF * QV].reshape(KT, 128, NQF, QV).transpose(2, 1, 0, 3)
    ).astype(WNP)
    VTC = VT if use_tail else QV
    thT_tail = np.zeros((H, VTC), dtype=np.float32)
    thT_tail[:, :VT] = thT[:, NQF * QV :]
    hwt = np.ascontiguousarray(
        thT_tail.reshape(KT, 128, VTC).transpose(1, 0, 2)
    ).astype(WNP)

    key = (tuple(sorted(cfg.items())), tuple(scales), head_scale, triv_ln,
           fp8_w, use_dr, use_tail)
    if key not in _NC_CACHE:
        _NC_CACHE.clear()
        nc = build_nc(cfg, scales, head_scale, triv_ln, fp8_w, use_dr,
                      use_tail)
        # Bacc.finalize runs the TRN2 legalization passes (1-wait-per-
        # instruction event-semaphore split, matmul->ldweights wait motion,
        # register allocation). The PJRT exec path serializes nc as-is.
        nc.finalize()
        _NC_CACHE[key] = nc
    nc = _NC_CACHE[key]

    common = {
        "w": wT,
        "hw": hw8,
        "hwt": hwt,
        "identb": np.eye(128, dtype=BF),
        "eps": np.full((128, 1), EPS, np.float32),
    }
    if not triv_ln:
        common.update(
            lng=ln_g.astype(BF),
            lnb=ln_b.astype(BF),
            lbias=layer_b.astype(BF),
            fing=final_g.astype(BF),
            finb=final_b.astype(BF),
        )
    in_maps = []
    for c in range(NC):
        h0c = np.ascontiguousarray(
            h0_full[c * TPC : (c + 1) * TPC].reshape(TT, 128, H)
        )
        # host-side pre-transpose of the layer-0 lhsT (scaled, bf16)
        h0Tc = np.ascontiguousarray(
            (h0c.reshape(TT, 128, KT, 128).transpose(0, 3, 2, 1)
             * np.float32(scales[0])).reshape(TT, 128, H)
        ).astype(BF)
        in_maps.append(dict(common, h0=h0c.astype(BF), h0T=h0Tc))

    trace = bool(int(os.environ.get("TRIKERNEL_TRACE", "0")))
    res = run_bass_kernel_spmd(nc, in_maps, core_ids=list(range(NC)), trace=trace)
    _LAST_RESULTS = res

    full = np.concatenate(
        [np.asarray(res.results[c]["out"]).astype(np.float32) for c in range(NC)],
        axis=0,
    )  # [NTOK, V]
    return full.reshape(2, 1024, 32000)
